# revision 1
# baseline (speedup 1.0000x reference)
"""BPR pairwise softplus loss on 8 Trainium2 NeuronCores.

loss = mean_b sum_{i<K, j>=K, both valid} softplus(pred[b,j] - pred[b,i])

Strategy (data parallel over batch, 32 rows/core), folding FOUR negatives
per ln via elementary symmetric polynomials:

  prod_{m=1..4} (1 + F*E_m) = 1 + F*c1 + F^2*c2 + F^3*c3 + F^4*c4
  =>  sum_m softplus(n_m - p) = ln(1 + sum_k F^k c_k),  F = exp(-p), E = exp(n)

The per-(pos, quad) evaluation is ONE matmul per row pair with contraction
dim 8 = (power k, row-of-pair b): stationary lhsT[(k,b), p] holds
interleave-masked F^k, moving rhs[(k,b), (u, j)] holds quad coefficients.
The matmul charges only output free size, so packing the powers into the
contraction dim quarters PE time vs. accumulation chains.

Invalid slots (target == -1) are folded into the prediction on the host
during sharding: invalid positives -> +50 (F = e^-50 -> 0), invalid
negatives -> -50 (E = e^-50 -> 0), so the device needs no target tensor,
no masking ops and no mask DMA.

Compute engines require all operands to start on the same partition, and a
DMA requires each side's partition coords to be a nested prefix of the
iteration order.  Both are satisfied by REPLICATING pred 4x across
partition blocks in the input DMA itself (stride-0 DRAM reads): partition
= 32*k + row.  Per-power ops then run on same-base block slices, and the
operand packs become plain tile-to-tile DMAs (flat-order reshapes).

  - ScalarE: exp over negatives (all replicas in one op), two F exps, two
    Ln(x+1) passes over PSUM with accum_out row sums.
  - VectorE: pair fold, per-block quad coeffs, interleave-mask selects.
  - Pool/GpSimd: F power chain, lhsT SWDGE pack.
  - per-partition partials DMA'd out; host sums 8x128xNPASS / B.
"""
import sys

sys.path.insert(0, "/opt/trn_rl_repo")

import numpy as np
import ml_dtypes

import concourse.bass as bass
import concourse.mybir as mybir
from concourse import bacc
import concourse.hw_specs as hw_specs
from concourse.tile import TileContext
from concourse.bass_utils import run_bass_kernel_spmd

B, N, K = 256, 512, 64
NC = 8
RPC = B // NC            # 32 batch rows per core
NPAIR = RPC // 2         # 16 row pairs (u paired with u+16)
NEG = N - K              # 448 negatives per row
G = 4                    # negatives folded per ln
NQ = NEG // G            # 112 quad groups per row
NPASS = 2                # Ln passes (2 PSUM banks each)
N_WARM = 65              # PE clock warm-up dummy matmuls
PRED_SPLIT = True        # split pred DMA into negs + pos
P_INPLACE = False       # build F powers by in-place block mults
POOLS4 = False           # four tile pools vs two
E_HN = 256               # width of the first exp chunk (asymmetric split)

_PROG_CACHE = {}

EXP = mybir.ActivationFunctionType.Exp
LN = mybir.ActivationFunctionType.Ln
F32 = mybir.dt.float32
BF16 = mybir.dt.bfloat16


def _patch_act_tables():
    """Make natural_log_exp_and_others the only table set advertising exp/ln
    so Bacc's table-load pass emits a single ACT_TABLE_LOAD."""
    if getattr(hw_specs.get_activation_tables, "_bpr_patched", False):
        return
    orig_fn = hw_specs.get_activation_tables

    def patched(arch):
        d = orig_fn(arch)
        out = {}
        for name, funcs in d.items():
            if name != "natural_log_exp_and_others" and (EXP in funcs
                                                         or LN in funcs):
                funcs = funcs - {EXP, LN}
            out[name] = funcs
        return out

    patched._bpr_patched = True
    hw_specs.get_activation_tables = patched
    bacc.get_activation_tables = patched


def build_program(nreps: int = 1):
    if nreps in _PROG_CACHE:
        return _PROG_CACHE[nreps]
    _patch_act_tables()
    nc = bacc.Bacc("TRN2", target_bir_lowering=False, debug=False,
                   num_devices=NC)
    pred = nc.dram_tensor("pred", [RPC, N], BF16, kind="ExternalInput")
    # interleave mask, replicated per power block: rows 32k+(0..15) keep
    # cols 0:64, rows 32k+(16..31) keep cols 64:128
    pm = nc.dram_tensor("pm", [4 * RPC, 2 * K], BF16, kind="ExternalInput")
    y = nc.dram_tensor("y", [nreps, 128, NPASS], F32, kind="ExternalOutput")

    mul = mybir.AluOpType.mult
    add = mybir.AluOpType.add

    from contextlib import ExitStack
    with TileContext(nc) as tc, ExitStack() as st:
        io = st.enter_context(tc.tile_pool(name="io", bufs=1))
        ps = st.enter_context(tc.tile_pool(name="ps", bufs=1, space="PSUM"))
        if POOLS4:
            mmp = st.enter_context(tc.tile_pool(name="mm", bufs=2))
            scr = st.enter_context(tc.tile_pool(name="scr", bufs=2))
        else:
            mmp = io
            scr = io
        if True:
            # Trigger the exp/ln activation-table load ASAP (~1.3us on
            # ScalarE, overlapping the input DMA).
            d0 = io.tile([128, 1], F32, tag="d0")
            nc.vector.memset(d0, 0.0)
            d1 = io.tile([128, 1], BF16, tag="d1")
            nc.scalar.activation(d1, d0, EXP)

            for rep in range(nreps):
                # negatives first on the fast SP queue (gates the exp
                # chain), positives separately
                pred_sb = io.tile([4 * RPC, N], BF16, tag="pred")
                if PRED_SPLIT:
                    nc.sync.dma_start(
                        out=pred_sb[:, K:N],
                        in_=pred[:, K:N].unsqueeze(0).broadcast_to(
                            [4, RPC, NEG]))
                    nc.sync.dma_start(
                        out=pred_sb[:, 0:K],
                        in_=pred[:, 0:K].unsqueeze(0).broadcast_to(
                            [4, RPC, K]))
                else:
                    nc.sync.dma_start(
                        out=pred_sb,
                        in_=pred[:].unsqueeze(0).broadcast_to([4, RPC, N]))
                pm_sb = io.tile([4 * RPC, 2 * K], BF16, tag="pm")
                nc.gpsimd.dma_start(out=pm_sb, in_=pm[:])

                # ---- E side (critical): E = exp(pred_neg), asymmetric
                # 256/192 split: the wider first chunk feeds wider first
                # folds, removing the DVE idle gap before the second ----
                HN = E_HN
                e_raw = io.tile([4 * RPC, NEG], BF16, tag="eraw")
                nc.scalar.activation(e_raw[:, 0:HN], pred_sb[:, K:K + HN],
                                     EXP)

                # F exp once; the duplicate half is a Pool copy
                fdup = io.tile([4 * RPC, 2 * K], BF16, tag="fdup")
                nc.scalar.activation(fdup[:, 0:K], pred_sb[:, 0:K], EXP,
                                     scale=-1.0)

                nc.scalar.activation(e_raw[:, HN:NEG], pred_sb[:, K + HN:N],
                                     EXP)
                nc.gpsimd.tensor_copy(fdup[:, K:2 * K], fdup[:, 0:K])

                # F^2 (and F^3 for the select variant) on Pool
                t2 = io.tile([4 * RPC, 2 * K], BF16, tag="t2")
                nc.gpsimd.tensor_tensor(t2, fdup, fdup, mul)
                if not P_INPLACE:
                    t3 = io.tile([4 * RPC, 2 * K], BF16, tag="t3")
                    nc.gpsimd.tensor_tensor(t3, t2, fdup, mul)

                # pair fold per half: pairs (x, x+112) within each half;
                # am = [aA | aB | mA | mB]; adds on DVE, mults on Pool so
                # the post-E-B DVE stream (which gates the rhs pack) is
                # shorter
                # pairs (x, x+128) within chunk A, (x, x+96) within chunk B
                am_a = io.tile([4 * RPC, 2 * NQ], BF16, tag="ama")
                am_m = io.tile([4 * RPC, 2 * NQ], BF16, tag="amm")
                HA = HN // 2
                nc.vector.tensor_tensor(am_a[:, 0:HA], e_raw[:, 0:HA],
                                        e_raw[:, HA:HN], add)
                nc.vector.tensor_tensor(am_m[:, 0:HA], e_raw[:, 0:HA],
                                        e_raw[:, HA:HN], mul)
                HB = (NEG - HN) // 2
                nc.vector.tensor_tensor(am_a[:, HA:HA + HB],
                                        e_raw[:, HN:HN + HB],
                                        e_raw[:, HN + HB:NEG], add)
                nc.vector.tensor_tensor(am_m[:, HA:HA + HB],
                                        e_raw[:, HN:HN + HB],
                                        e_raw[:, HN + HB:NEG], mul)

                # quad coeffs per power block: c1 = a1+a2,
                # c2 = m1+m2+a1*a2, c3 = a1*m2+a2*m1, c4 = m1*m2
                # (quads {x, x+112, x+224, x+336})
                q = io.tile([4 * RPC, NQ], BF16, tag="q")
                t1 = scr.tile([4 * RPC, NQ], BF16, tag="t1")
                R1, R2, R3, R4 = RPC, 2 * RPC, 3 * RPC, 4 * RPC
                A1 = slice(0, NQ)
                A2 = slice(NQ, 2 * NQ)
                # am_m (Pool) lands later than am_a: mult-free ops first
                nc.vector.tensor_tensor(q[0:R1], am_a[0:R1, A1],
                                        am_a[0:R1, A2], add)
                nc.vector.tensor_tensor(t1[R1:R2], am_a[R1:R2, A1],
                                        am_a[R1:R2, A2], mul)
                nc.vector.tensor_tensor(t1[R2:R3], am_a[R2:R3, A1],
                                        am_m[R2:R3, A2], mul)
                nc.vector.tensor_tensor(q[R2:R3], am_a[R2:R3, A2],
                                        am_m[R2:R3, A1], mul)
                nc.vector.tensor_tensor(q[R1:R2], am_m[R1:R2, A1],
                                        am_m[R1:R2, A2], add)
                nc.vector.tensor_tensor(q[R1:R2], q[R1:R2], t1[R1:R2], add)
                nc.vector.tensor_tensor(q[R2:R3], q[R2:R3], t1[R2:R3], add)
                nc.vector.tensor_tensor(q[R3:R4], am_m[R3:R4, A1],
                                        am_m[R3:R4, A2], mul)

                # pack quad coeffs: rhs[(k,b), (u,j)] - flat reshape, on the
                # fast SP HWDGE queue, issued as soon as the quads land
                rhs = mmp.tile([2 * G, NQ * NPAIR], BF16, tag="rhs")
                nc.sync.dma_start(out=rhs, in_=q)

                # interleave-masked F powers
                P = io.tile([4 * RPC, 2 * K], BF16, tag="P")
                if P_INPLACE:
                    # pm is 0/1 so pm^2 = pm: P = F*pm everywhere, then
                    # block k *= F^(k-1) factors in place
                    nc.vector.tensor_tensor(P, fdup, pm_sb, mul)
                    nc.vector.tensor_tensor(P[R1:R2], P[R1:R2],
                                            fdup[R1:R2], mul)
                    nc.vector.tensor_tensor(P[R2:R3], P[R2:R3], t2[R2:R3],
                                            mul)
                    nc.vector.tensor_tensor(P[R3:R4], P[R3:R4],
                                            fdup[R3:R4], mul)
                    nc.vector.tensor_tensor(P[R3:R4], P[R3:R4], t2[R3:R4],
                                            mul)
                else:
                    t4 = io.tile([4 * RPC, 2 * K], BF16, tag="t4")
                    nc.vector.tensor_tensor(t4, t2, t2, mul)
                    nc.vector.tensor_tensor(P[0:R1], fdup[0:R1],
                                            pm_sb[0:R1], mul)
                    nc.vector.tensor_tensor(P[R1:R2], t2[R1:R2],
                                            pm_sb[R1:R2], mul)
                    nc.vector.tensor_tensor(P[R3:R4], t4[R3:R4],
                                            pm_sb[R3:R4], mul)
                    nc.vector.tensor_tensor(P[R2:R3], t3[R2:R3],
                                            pm_sb[R2:R3], mul)

                # pack F powers: lhsT[(k,b), (u,p)] - flat reshape, second
                # SP HWDGE transfer (pipelines behind rhs)
                lhsT = mmp.tile([2 * G, 2 * K * NPAIR], BF16, tag="lhsT")
                nc.sync.dma_start(out=lhsT, in_=P)

                # one matmul per rowpair u: psum[p, j] = sum_k F^k c_k
                # slot(u) = 512*(u//4) + 112*(u%4); 2 banks per Ln pass,
                # separate tiles so each Ln pass depends only on its half
                pt0 = ps.tile([128, 2 * 512], F32, tag="ps0")
                pt1 = ps.tile([128, 2 * 512], F32, tag="ps1")
                pts = [pt0, pt1]

                # warm matmuls gated on the early pm DMA keep the PE clock
                # ramping from ~3us so the real stream runs at full speed
                # (they write pad columns of the psum tiles)
                for w in range(N_WARM):
                    nc.tensor.matmul(pt0[:, 448:512], pm_sb[0:2, 0:2 * K],
                                     pm_sb[0:2, 0:K], start=True, stop=True)
                for u in range(NPAIR):
                    pt = pts[u // 8]
                    uu = u % 8
                    out_sl = pt[:, 512 * (uu // 4) + NQ * (uu % 4):
                                512 * (uu // 4) + NQ * (uu % 4) + NQ]
                    nc.tensor.matmul(out_sl,
                                     lhsT[:, 2 * K * u: 2 * K * (u + 1)],
                                     rhs[:, NQ * u: NQ * (u + 1)],
                                     start=True, stop=True)

                # ln(1 + psum), accumulated per partition; NPASS passes of
                # 2 banks each.  The throwaway ln outputs land in spare
                # PSUM banks: PSUM access init (172 cyc) < SBUF (222 cyc),
                # trimming each pass.
                partials = mmp.tile([128, NPASS], F32, tag="part")
                sout = ps.tile([128, 2 * 512], F32, tag="souts")
                for i in range(NPASS):
                    nc.scalar.activation(
                        sout.rearrange("p (b x) -> p b x",
                                       x=512)[:, :, 0:4 * NQ],
                        pts[i].rearrange("p (b x) -> p b x",
                                         x=512)[:, :, 0:4 * NQ],
                        LN, bias=1.0,
                        accum_out=partials[:, i:i + 1])

                nc.sync.dma_start(out=y[rep], in_=partials)

    nc.finalize()
    _PROG_CACHE[nreps] = (nc, ())
    return nc, ()


def _pm_const():
    pmv = np.zeros((4 * RPC, 2 * K), dtype=ml_dtypes.bfloat16)
    for k in range(4):
        pmv[32 * k:32 * k + NPAIR, 0:K] = 1
        pmv[32 * k + NPAIR:32 * k + RPC, K:2 * K] = 1
    return pmv


def make_in_maps(prediction, target, consts):
    # fold validity into the prediction: invalid positives -> +50
    # (F = e^-50 -> 0), invalid negatives -> -50 (E = e^-50 -> 0)
    fill = np.empty((1, N), np.float32)
    fill[:, 0:K] = 50.0
    fill[:, K:N] = -50.0
    pred_m = np.where(target == -1, fill,
                      prediction).astype(ml_dtypes.bfloat16)
    pmv = _pm_const()
    in_maps = []
    for c in range(NC):
        in_maps.append({
            "pred": np.ascontiguousarray(pred_m[c * RPC:(c + 1) * RPC]),
            "pm": pmv,
        })
    return in_maps


def kernel(prediction, target):
    nc, consts = build_program(1)
    in_maps = make_in_maps(prediction, target, consts)
    res = run_bass_kernel_spmd(nc, in_maps, core_ids=list(range(NC)))
    total = sum(float(res.results[c]["y"][0].sum(dtype=np.float64))
                for c in range(NC))
    return np.float32(total / B)



# revision 12
# speedup vs baseline: 1.1894x; 1.1894x over previous
"""BPR pairwise softplus loss on 8 Trainium2 NeuronCores.

loss = mean_b sum_{i<K, j>=K, both valid} softplus(pred[b,j] - pred[b,i])

Strategy (data parallel over batch, 32 rows/core), folding FOUR negatives
per ln via elementary symmetric polynomials:

  prod_{m=1..4} (1 + F*E_m) = 1 + F*c1 + F^2*c2 + F^3*c3 + F^4*c4
  =>  sum_m softplus(n_m - p) = ln(1 + sum_k F^k c_k),  F = exp(-p), E = exp(n)

Pack-free layout: partition = 4*r + k (row-major, power k innermost), so the
8 contraction partitions of row pair t = (2t, 2t+1) are the contiguous range
8t..8t+8 and the matmul reads the power tile P and coefficient tile q
DIRECTLY - no SBUF->SBUF pack DMAs (each DMA hop costs ~2.7us of fixed
latency: 565 SEQ + 625 HWDGE + 650 engine delay + transfer + 900 sem).

  - P_v[4r+k, 64h+p] = F^{k+1}[r, p] masked to half h == r%2 AND to pairs
    with (r//2)%4 == v: each P_v is ONE ScalarE exp with per-partition
    scale -(k+1) / bias 0 on live partitions and scale 0 / bias -100
    (exp -> 0) elsewhere; the column-interleave mask is folded into the
    host data (+50 fill -> exp(-(k+1)*50) = 0).
  - q[4r+k, j] = c_{k+1}[r, j]: pair folds a/m then masked placement with
    per-partition 0/1 scalar masks via scalar_tensor_tensor:
      q = M1*(a1+a2) + M2*(m1+m2+a1*a2) + M3*(a1*m2+a2*m1) + M4*(m1*m2)
  - 16 matmuls (contraction 32 = one aligned block of 4 pairs, lhsT P_v
    zeroing the other 3 pairs, free 112) straight into PSUM; asymmetric Ln
    passes (N0 pairs then 16-N0) with accum_out row sums; one output DMA.
    (PE tiling: operands must sit at partition base 0/32/64/96 with an
    explicit tile_position, hence the 32-block contraction.)
  - Dummy warm matmuls from ~1us keep the PE p-state ramping so the real
    matmuls run at full clock (a PE idle gap resets the 3us ramp).

Invalid slots (target == -1) fold into the prediction on the host: invalid
positives -> +50 (F^k -> 0), invalid negatives -> -50 (E -> 0).
"""
import sys

sys.path.insert(0, "/opt/trn_rl_repo")

import numpy as np
import ml_dtypes

import concourse.bass as bass
import concourse.mybir as mybir
from concourse import bacc
import concourse.hw_specs as hw_specs
from concourse.tile import TileContext
from concourse.bass_utils import run_bass_kernel_spmd

B, N, K = 256, 512, 64
NC = 8
RPC = B // NC            # 32 batch rows per core
NPAIR = RPC // 2         # 16 row pairs (2t, 2t+1)
NEG = N - K              # 448 negatives per row
G = 4                    # negatives folded per ln
NQ = NEG // G            # 112 quad groups per row
NPASS = 2                # Ln passes
N0 = 4                   # row pairs in Ln pass 0 (rest in pass 1); multiple
                         # of 4 so each pass covers whole PSUM banks
N_WARM = 440             # PE clock warm-up dummy matmuls
WARM_COLS = 16           # free size of each warm matmul

_PROG_CACHE = {}

EXP = mybir.ActivationFunctionType.Exp
LN = mybir.ActivationFunctionType.Ln
F32 = mybir.dt.float32
BF16 = mybir.dt.bfloat16


def _patch_act_tables():
    """Make natural_log_exp_and_others the only table set advertising exp/ln
    so Bacc's table-load pass emits a single ACT_TABLE_LOAD."""
    if getattr(hw_specs.get_activation_tables, "_bpr_patched", False):
        return
    orig_fn = hw_specs.get_activation_tables

    def patched(arch):
        d = orig_fn(arch)
        out = {}
        for name, funcs in d.items():
            if name != "natural_log_exp_and_others" and (EXP in funcs
                                                         or LN in funcs):
                funcs = funcs - {EXP, LN}
            out[name] = funcs
        return out

    patched._bpr_patched = True
    hw_specs.get_activation_tables = patched
    bacc.get_activation_tables = patched


def build_program(nreps: int = 1):
    if nreps in _PROG_CACHE:
        return _PROG_CACHE[nreps]
    _patch_act_tables()
    nc = bacc.Bacc("TRN2", target_bir_lowering=False, debug=False,
                   num_devices=NC)
    negs_d = nc.dram_tensor("negs", [RPC, NEG], BF16, kind="ExternalInput")
    # pre-masked doubled positives: [r, 64h+p] = pred_pos[r,p] if h==r%2
    # else +50
    posd_d = nc.dram_tensor("posd", [RPC, 2 * K], BF16, kind="ExternalInput")
    # cols 0-3: P_v exp scales; cols 4-7: P_v exp biases; cols 8-11: one-hot
    # coefficient masks M1..M4 (p%4 == k)
    consts_d = nc.dram_tensor("consts", [128, 16], F32, kind="ExternalInput")
    y = nc.dram_tensor("y", [nreps, 128, NPASS], F32, kind="ExternalOutput")

    mul = mybir.AluOpType.mult
    add = mybir.AluOpType.add

    from contextlib import ExitStack
    with TileContext(nc) as tc, ExitStack() as st:
        io = st.enter_context(tc.tile_pool(name="io", bufs=1))
        ps = st.enter_context(tc.tile_pool(name="ps", bufs=1, space="PSUM"))

        # Trigger the exp/ln activation-table load ASAP (~1.3us on ScalarE,
        # overlapping the input DMA).
        d0 = io.tile([128, 1], F32, tag="d0")
        nc.vector.memset(d0, 0.0)
        d1 = io.tile([128, 1], BF16, tag="d1")
        nc.scalar.activation(d1, d0, EXP)

        # dummy operands for the PE warm-up chain
        dwr = io.tile([8, WARM_COLS], BF16, tag="dwr")
        nc.vector.memset(dwr, 0.0)

        for rep in range(nreps):
            # consts on the Pool SWDGE queue (parallel with SP inputs)
            cs = io.tile([128, 16], F32, tag="cs")
            nc.gpsimd.dma_start(out=cs, in_=consts_d[:])
            m1 = cs[:, 8:9]
            m2 = cs[:, 9:10]
            m3 = cs[:, 10:11]
            m4 = cs[:, 11:12]

            # inputs, replicated 4x across power slots (partition = 4r+k)
            # via stride-0 DRAM reads; negatives first (they gate the chain)
            negs = io.tile([128, NEG], BF16, tag="negs")
            nc.sync.dma_start(
                out=negs,
                in_=negs_d[:].unsqueeze(1).broadcast_to([RPC, 4, NEG]))
            posd = io.tile([128, 2 * K], BF16, tag="posd")
            nc.sync.dma_start(
                out=posd,
                in_=posd_d[:].unsqueeze(1).broadcast_to([RPC, 4, 2 * K]))

            # PE warm-up: keep the clock ramping from ~1us until the real
            # matmuls (psum scratch, no consumers)
            pw = ps.tile([128, WARM_COLS], F32, tag="pw")
            for w in range(N_WARM):
                nc.tensor.matmul(pw[0:WARM_COLS], dwr, dwr,
                                 start=True, stop=True)

            # ---- ScalarE stream: exp chunks C1/C2 over negatives, then the
            # four pair-masked power tiles P_v = exp(posd*scale_v + bias_v)
            HC = NEG // 2
            e = io.tile([128, NEG], BF16, tag="e")
            nc.scalar.activation(e[:, 0:HC], negs[:, 0:HC], EXP)
            nc.scalar.activation(e[:, HC:NEG], negs[:, HC:NEG], EXP)
            pv = []
            for v in range(4):
                pvt = io.tile([128, 2 * K], BF16, tag=f"P{v}")
                nc.scalar.activation(pvt, posd, EXP, scale=cs[:, v:v + 1],
                                     bias=cs[:, 4 + v:5 + v])
                pv.append(pvt)

            # ---- DVE: pair folds (x, x+112 within each 224 chunk) ----
            a = io.tile([128, 2 * NQ], BF16, tag="a")
            m = io.tile([128, 2 * NQ], BF16, tag="m")
            nc.vector.tensor_tensor(a[:, 0:NQ], e[:, 0:NQ], e[:, NQ:2 * NQ],
                                    add)
            nc.vector.tensor_tensor(m[:, 0:NQ], e[:, 0:NQ], e[:, NQ:2 * NQ],
                                    mul)
            nc.vector.tensor_tensor(a[:, NQ:2 * NQ], e[:, 2 * NQ:3 * NQ],
                                    e[:, 3 * NQ:4 * NQ], add)
            nc.vector.tensor_tensor(m[:, NQ:2 * NQ], e[:, 2 * NQ:3 * NQ],
                                    e[:, 3 * NQ:4 * NQ], mul)
            a1 = a[:, 0:NQ]
            a2 = a[:, NQ:2 * NQ]
            mm1 = m[:, 0:NQ]
            mm2 = m[:, NQ:2 * NQ]

            # ---- masked coefficient placement:
            # q[4r+k] = c_{k+1}[r]; Mk are per-partition 0/1 scalars ----
            w = io.tile([128, NQ], BF16, tag="w")
            v = io.tile([128, NQ], BF16, tag="v")
            # two independent products on Pool (off the DVE critical path);
            # Pool only supports plain tensor_tensor, masking happens in the
            # DVE stt chain below
            nc.gpsimd.tensor_tensor(w, a2, mm1, mul)
            nc.gpsimd.tensor_tensor(v, mm1, mm2, mul)

            s_a = io.tile([128, NQ], BF16, tag="sa")
            t_aa2 = io.tile([128, NQ], BF16, tag="taa2")
            s_m = io.tile([128, NQ], BF16, tag="sm")
            t_am3 = io.tile([128, NQ], BF16, tag="tam3")
            y1 = io.tile([128, NQ], BF16, tag="y1")
            y2 = io.tile([128, NQ], BF16, tag="y2")
            y3 = io.tile([128, NQ], BF16, tag="y3")
            y4 = io.tile([128, NQ], BF16, tag="y4")
            q = io.tile([128, NQ], BF16, tag="q")
            nc.vector.tensor_tensor(s_a, a1, a2, add)
            nc.vector.scalar_tensor_tensor(t_aa2, a1, m2, a2, mul, mul)
            nc.vector.tensor_tensor(s_m, mm1, mm2, add)
            nc.vector.scalar_tensor_tensor(t_am3, mm2, m3, a1, mul, mul)
            nc.vector.scalar_tensor_tensor(y1, s_a, m1, t_aa2, mul, add)
            nc.vector.scalar_tensor_tensor(y2, s_m, m2, t_am3, mul, add)
            nc.vector.tensor_tensor(y3, y1, y2, add)
            nc.vector.scalar_tensor_tensor(y4, w, m3, y3, mul, add)
            nc.vector.scalar_tensor_tensor(q, v, m4, y4, mul, add)

            # ---- one matmul per row pair t: psum[64h+p, j] =
            # sum_k F^{k+1}[2t+h, p] * c_{k+1}[2t+h, j]; contraction is the
            # aligned 32-block of 4 pairs, P_{t%4} zeroes the other 3 ----
            # psum: 4 pairs per 512-col bank (a matmul output cannot cross a
            # bank boundary); pass 0 = bank 0, pass 1 = banks 1..3
            NB1 = (NPAIR - N0) // 4
            pt0 = ps.tile([128, 512], F32, tag="ps0")
            pt1 = ps.tile([128, NB1 * 512], F32, tag="ps1")
            for t in range(NPAIR):
                if t < N0:
                    out_sl = pt0[:, NQ * t:NQ * (t + 1)]
                else:
                    u = t - N0
                    base = 512 * (u // 4) + NQ * (u % 4)
                    out_sl = pt1[:, base:base + NQ]
                blk = 32 * (t // 4)
                nc.tensor.matmul(out_sl,
                                 pv[t % 4][blk:blk + 32, :],
                                 q[blk:blk + 32, :],
                                 start=True, stop=True,
                                 tile_position=(blk, 0))

            # ---- ln(1 + psum), accumulated per partition; asymmetric
            # passes so pass 0 starts after only N0 pairs of matmuls.
            # Throwaway ln outputs land in spare PSUM (access init 172 cyc
            # < SBUF 222). ----
            partials = io.tile([128, NPASS], F32, tag="part")
            sout = ps.tile([128, (NPAIR - N0) * NQ], F32, tag="souts")
            nc.scalar.activation(sout[:, 0:N0 * NQ], pt0[:, 0:N0 * NQ], LN,
                                 bias=1.0, accum_out=partials[:, 0:1])
            nc.scalar.activation(
                sout[:, 0:(NPAIR - N0) * NQ].rearrange(
                    "p (b x) -> p b x", x=4 * NQ),
                pt1.rearrange("p (b x) -> p b x", x=512)[:, :, 0:4 * NQ],
                LN, bias=1.0, accum_out=partials[:, 1:2])

            nc.sync.dma_start(out=y[rep], in_=partials)

    nc.finalize()
    _PROG_CACHE[nreps] = (nc, ())
    return nc, ()


def _consts():
    cs = np.zeros((128, 16), dtype=np.float32)
    p = np.arange(128)
    k = p % 4
    vblk = (p % 32) // 8  # which pair-of-the-32-block this partition feeds
    for v in range(4):
        live = vblk == v
        cs[:, v] = np.where(live, -(1.0 + k), 0.0)
        cs[:, 4 + v] = np.where(live, 0.0, -100.0)
    for i in range(4):
        cs[:, 8 + i] = (k == i).astype(np.float32)
    return cs


def make_in_maps(prediction, target, consts):
    # fold validity into the prediction: invalid positives -> +50
    # (F^k = e^{-50k} -> 0), invalid negatives -> -50 (E = e^-50 -> 0)
    fill = np.empty((1, N), np.float32)
    fill[:, 0:K] = 50.0
    fill[:, K:N] = -50.0
    pred_m = np.where(target == -1, fill, prediction).astype(np.float32)
    csv = _consts()
    in_maps = []
    for c in range(NC):
        blk = pred_m[c * RPC:(c + 1) * RPC]
        negs = np.ascontiguousarray(blk[:, K:N]).astype(ml_dtypes.bfloat16)
        # doubled positives with the pair-interleave mask folded in:
        # posd[r, 64h:64h+64] = pos row r if h == r%2 else +50
        posd = np.full((RPC, 2, K), 50.0, np.float32)
        rr = np.arange(RPC)
        posd[rr, rr % 2, :] = blk[:, 0:K]
        in_maps.append({
            "negs": negs,
            "posd": posd.reshape(RPC, 2 * K).astype(ml_dtypes.bfloat16),
            "consts": csv,
        })
    return in_maps


def kernel(prediction, target):
    nc, consts = build_program(1)
    in_maps = make_in_maps(prediction, target, consts)
    res = run_bass_kernel_spmd(nc, in_maps, core_ids=list(range(NC)))
    total = sum(float(res.results[c]["y"][0].sum(dtype=np.float64))
                for c in range(NC))
    return np.float32(total / B)


# revision 13
# speedup vs baseline: 1.1918x; 1.0020x over previous
"""BPR pairwise softplus loss on 8 Trainium2 NeuronCores.

loss = mean_b sum_{i<K, j>=K, both valid} softplus(pred[b,j] - pred[b,i])

Strategy (data parallel over batch, 32 rows/core), folding FOUR negatives
per ln via elementary symmetric polynomials:

  prod_{m=1..4} (1 + F*E_m) = 1 + F*c1 + F^2*c2 + F^3*c3 + F^4*c4
  =>  sum_m softplus(n_m - p) = ln(1 + sum_k F^k c_k),  F = exp(-p), E = exp(n)

Pack-free layout: partition = 4*r + k (row-major, power k innermost), so the
8 contraction partitions of row pair t = (2t, 2t+1) are the contiguous range
8t..8t+8 and the matmul reads the power tile P and coefficient tile q
DIRECTLY - no SBUF->SBUF pack DMAs (each DMA hop costs ~2.7us of fixed
latency: 565 SEQ + 625 HWDGE + 650 engine delay + transfer + 900 sem).

  - P_v[4r+k, 64h+p] = F^{k+1}[r, p] masked to half h == r%2 AND to pairs
    with (r//2)%4 == v: each P_v is ONE ScalarE exp with per-partition
    scale -(k+1) / bias 0 on live partitions and scale 0 / bias -100
    (exp -> 0) elsewhere; the column-interleave mask is folded into the
    host data (+50 fill -> exp(-(k+1)*50) = 0).
  - q[4r+k, j] = c_{k+1}[r, j]: pair folds a/m then masked placement with
    per-partition 0/1 scalar masks via scalar_tensor_tensor:
      q = M1*(a1+a2) + M2*(m1+m2+a1*a2) + M3*(a1*m2+a2*m1) + M4*(m1*m2)
  - 16 matmuls (contraction 32 = one aligned block of 4 pairs, lhsT P_v
    zeroing the other 3 pairs, free 112) straight into PSUM; asymmetric Ln
    passes (N0 pairs then 16-N0) with accum_out row sums; one output DMA.
    (PE tiling: operands must sit at partition base 0/32/64/96 with an
    explicit tile_position, hence the 32-block contraction.)
  - Dummy warm matmuls from ~1us keep the PE p-state ramping so the real
    matmuls run at full clock (a PE idle gap resets the 3us ramp).

Invalid slots (target == -1) fold into the prediction on the host: invalid
positives -> +50 (F^k -> 0), invalid negatives -> -50 (E -> 0).
"""
import sys

sys.path.insert(0, "/opt/trn_rl_repo")

import numpy as np
import ml_dtypes

import concourse.bass as bass
import concourse.mybir as mybir
from concourse import bacc
import concourse.hw_specs as hw_specs
from concourse.tile import TileContext
from concourse.bass_utils import run_bass_kernel_spmd

B, N, K = 256, 512, 64
NC = 8
RPC = B // NC            # 32 batch rows per core
NPAIR = RPC // 2         # 16 row pairs (2t, 2t+1)
NEG = N - K              # 448 negatives per row
G = 4                    # negatives folded per ln
NQ = NEG // G            # 112 quad groups per row
NPASS = 2                # Ln passes
N0 = 4                   # row pairs in Ln pass 0 (rest in pass 1); multiple
                         # of 4 so each pass covers whole PSUM banks
N_WARM = 440             # PE clock warm-up dummy matmuls
WARM_COLS = 16           # free size of each warm matmul

_PROG_CACHE = {}

EXP = mybir.ActivationFunctionType.Exp
LN = mybir.ActivationFunctionType.Ln
F32 = mybir.dt.float32
BF16 = mybir.dt.bfloat16


def _patch_act_tables():
    """Make natural_log_exp_and_others the only table set advertising exp/ln
    so Bacc's table-load pass emits a single ACT_TABLE_LOAD."""
    if getattr(hw_specs.get_activation_tables, "_bpr_patched", False):
        return
    orig_fn = hw_specs.get_activation_tables

    def patched(arch):
        d = orig_fn(arch)
        out = {}
        for name, funcs in d.items():
            if name != "natural_log_exp_and_others" and (EXP in funcs
                                                         or LN in funcs):
                funcs = funcs - {EXP, LN}
            out[name] = funcs
        return out

    patched._bpr_patched = True
    hw_specs.get_activation_tables = patched
    bacc.get_activation_tables = patched


def build_program(nreps: int = 1):
    if nreps in _PROG_CACHE:
        return _PROG_CACHE[nreps]
    _patch_act_tables()
    nc = bacc.Bacc("TRN2", target_bir_lowering=False, debug=False,
                   num_devices=NC)
    negs_d = nc.dram_tensor("negs", [RPC, NEG], BF16, kind="ExternalInput")
    # pre-masked doubled positives: [r, 64h+p] = pred_pos[r,p] if h==r%2
    # else +50
    posd_d = nc.dram_tensor("posd", [RPC, 2 * K], BF16, kind="ExternalInput")
    # cols 0-3: P_v exp scales; cols 4-7: P_v exp biases; cols 8-11: one-hot
    # coefficient masks M1..M4 (p%4 == k)
    consts_d = nc.dram_tensor("consts", [128, 16], F32, kind="ExternalInput")
    y = nc.dram_tensor("y", [nreps, 128, NPASS], F32, kind="ExternalOutput")

    mul = mybir.AluOpType.mult
    add = mybir.AluOpType.add

    from contextlib import ExitStack
    with TileContext(nc) as tc, ExitStack() as st:
        io = st.enter_context(tc.tile_pool(name="io", bufs=1))
        ps = st.enter_context(tc.tile_pool(name="ps", bufs=1, space="PSUM"))

        # Trigger the exp/ln activation-table load ASAP (~1.3us on ScalarE,
        # overlapping the input DMA).
        d0 = io.tile([128, 1], F32, tag="d0")
        nc.vector.memset(d0, 0.0)
        d1 = io.tile([128, 1], BF16, tag="d1")
        nc.scalar.activation(d1, d0, EXP)

        # dummy operands for the PE warm-up chain
        dwr = io.tile([8, WARM_COLS], BF16, tag="dwr")
        nc.vector.memset(dwr, 0.0)

        for rep in range(nreps):
            # consts on the Pool SWDGE queue (parallel with SP inputs)
            cs = io.tile([128, 16], F32, tag="cs")
            nc.gpsimd.dma_start(out=cs, in_=consts_d[:])
            m1 = cs[:, 8:9]
            m2 = cs[:, 9:10]
            m3 = cs[:, 10:11]
            m4 = cs[:, 11:12]

            # inputs, replicated 4x across power slots (partition = 4r+k)
            # via stride-0 DRAM reads; negatives first (they gate the chain)
            negs = io.tile([128, NEG], BF16, tag="negs")
            nc.sync.dma_start(
                out=negs,
                in_=negs_d[:].unsqueeze(1).broadcast_to([RPC, 4, NEG]))
            posd = io.tile([128, 2 * K], BF16, tag="posd")
            nc.sync.dma_start(
                out=posd,
                in_=posd_d[:].unsqueeze(1).broadcast_to([RPC, 4, 2 * K]))

            # PE warm-up: keep the clock ramping from ~1us until the real
            # matmuls (psum scratch, no consumers)
            pw = ps.tile([128, WARM_COLS], F32, tag="pw")
            for w in range(N_WARM):
                nc.tensor.matmul(pw[0:WARM_COLS], dwr, dwr,
                                 start=True, stop=True)

            # ---- DVE early window (negs land long before exp finishes):
            # log-space sums so the product folds become ACT exps:
            #   m1 = E1*E2 = exp(n1+n2), m2 = E3*E4 = exp(n3+n4),
            #   v = m1*m2 = exp(n1+n2+n3+n4)
            # f32 sums keep exp() accurate (bf16 sums cost ~3% on exp) ----
            nsum = io.tile([128, 2 * NQ], F32, tag="nsum")
            nsq = io.tile([128, NQ], F32, tag="nsq")
            nc.vector.tensor_tensor(nsum[:, 0:NQ], negs[:, 0:NQ],
                                    negs[:, NQ:2 * NQ], add)
            nc.vector.tensor_tensor(nsum[:, NQ:2 * NQ],
                                    negs[:, 2 * NQ:3 * NQ],
                                    negs[:, 3 * NQ:4 * NQ], add)
            nc.vector.tensor_tensor(nsq, nsum[:, 0:NQ], nsum[:, NQ:2 * NQ],
                                    add)

            # ---- ScalarE stream: exp chunks C1/C2 over negatives, product
            # folds M/V from the log sums, then the four pair-masked power
            # tiles P_v = exp(posd*scale_v + bias_v) ----
            HC = NEG // 2
            e = io.tile([128, NEG], BF16, tag="e")
            nc.scalar.activation(e[:, 0:HC], negs[:, 0:HC], EXP)
            nc.scalar.activation(e[:, HC:NEG], negs[:, HC:NEG], EXP)
            m = io.tile([128, 2 * NQ], BF16, tag="m")
            nc.scalar.activation(m, nsum, EXP)
            v = io.tile([128, NQ], BF16, tag="v")
            nc.scalar.activation(v, nsq, EXP)
            pv = []
            for vi in range(4):
                pvt = io.tile([128, 2 * K], BF16, tag=f"P{vi}")
                nc.scalar.activation(pvt, posd, EXP, scale=cs[:, vi:vi + 1],
                                     bias=cs[:, 4 + vi:5 + vi])
                pv.append(pvt)

            # ---- DVE: additive pair folds ----
            a = io.tile([128, 2 * NQ], BF16, tag="a")
            nc.vector.tensor_tensor(a[:, 0:NQ], e[:, 0:NQ], e[:, NQ:2 * NQ],
                                    add)
            nc.vector.tensor_tensor(a[:, NQ:2 * NQ], e[:, 2 * NQ:3 * NQ],
                                    e[:, 3 * NQ:4 * NQ], add)
            a1 = a[:, 0:NQ]
            a2 = a[:, NQ:2 * NQ]
            mm1 = m[:, 0:NQ]
            mm2 = m[:, NQ:2 * NQ]

            # ---- masked coefficient placement:
            # q[4r+k] = c_{k+1}[r]; Mk are per-partition 0/1 scalars ----
            w = io.tile([128, NQ], BF16, tag="w")
            # one product on Pool (off the DVE critical path); Pool only
            # supports plain tensor_tensor, masking happens in the DVE stt
            # chain below
            nc.gpsimd.tensor_tensor(w, a2, mm1, mul)

            s_a = io.tile([128, NQ], BF16, tag="sa")
            t_aa2 = io.tile([128, NQ], BF16, tag="taa2")
            s_m = io.tile([128, NQ], BF16, tag="sm")
            t_am3 = io.tile([128, NQ], BF16, tag="tam3")
            y1 = io.tile([128, NQ], BF16, tag="y1")
            y2 = io.tile([128, NQ], BF16, tag="y2")
            y3 = io.tile([128, NQ], BF16, tag="y3")
            y4 = io.tile([128, NQ], BF16, tag="y4")
            q = io.tile([128, NQ], BF16, tag="q")
            nc.vector.tensor_tensor(s_a, a1, a2, add)
            nc.vector.scalar_tensor_tensor(t_aa2, a1, m2, a2, mul, mul)
            nc.vector.tensor_tensor(s_m, mm1, mm2, add)
            nc.vector.scalar_tensor_tensor(t_am3, mm2, m3, a1, mul, mul)
            nc.vector.scalar_tensor_tensor(y1, s_a, m1, t_aa2, mul, add)
            nc.vector.scalar_tensor_tensor(y2, s_m, m2, t_am3, mul, add)
            nc.vector.tensor_tensor(y3, y1, y2, add)
            nc.vector.scalar_tensor_tensor(y4, w, m3, y3, mul, add)
            nc.vector.scalar_tensor_tensor(q, v, m4, y4, mul, add)

            # ---- one matmul per row pair t: psum[64h+p, j] =
            # sum_k F^{k+1}[2t+h, p] * c_{k+1}[2t+h, j]; contraction is the
            # aligned 32-block of 4 pairs, P_{t%4} zeroes the other 3 ----
            # psum: 4 pairs per 512-col bank (a matmul output cannot cross a
            # bank boundary); pass 0 = bank 0, pass 1 = banks 1..3
            NB1 = (NPAIR - N0) // 4
            pt0 = ps.tile([128, 512], F32, tag="ps0")
            pt1 = ps.tile([128, NB1 * 512], F32, tag="ps1")
            for t in range(NPAIR):
                if t < N0:
                    out_sl = pt0[:, NQ * t:NQ * (t + 1)]
                else:
                    u = t - N0
                    base = 512 * (u // 4) + NQ * (u % 4)
                    out_sl = pt1[:, base:base + NQ]
                blk = 32 * (t // 4)
                nc.tensor.matmul(out_sl,
                                 pv[t % 4][blk:blk + 32, :],
                                 q[blk:blk + 32, :],
                                 start=True, stop=True,
                                 tile_position=(blk, 0))

            # ---- ln(1 + psum), accumulated per partition; asymmetric
            # passes so pass 0 starts after only N0 pairs of matmuls.
            # Throwaway ln outputs land in spare PSUM (access init 172 cyc
            # < SBUF 222). ----
            partials = io.tile([128, NPASS], F32, tag="part")
            sout = ps.tile([128, (NPAIR - N0) * NQ], F32, tag="souts")
            nc.scalar.activation(sout[:, 0:N0 * NQ], pt0[:, 0:N0 * NQ], LN,
                                 bias=1.0, accum_out=partials[:, 0:1])
            nc.scalar.activation(
                sout[:, 0:(NPAIR - N0) * NQ].rearrange(
                    "p (b x) -> p b x", x=4 * NQ),
                pt1.rearrange("p (b x) -> p b x", x=512)[:, :, 0:4 * NQ],
                LN, bias=1.0, accum_out=partials[:, 1:2])

            nc.sync.dma_start(out=y[rep], in_=partials)

    nc.finalize()
    _PROG_CACHE[nreps] = (nc, ())
    return nc, ()


def _consts():
    cs = np.zeros((128, 16), dtype=np.float32)
    p = np.arange(128)
    k = p % 4
    vblk = (p % 32) // 8  # which pair-of-the-32-block this partition feeds
    for v in range(4):
        live = vblk == v
        cs[:, v] = np.where(live, -(1.0 + k), 0.0)
        cs[:, 4 + v] = np.where(live, 0.0, -100.0)
    for i in range(4):
        cs[:, 8 + i] = (k == i).astype(np.float32)
    return cs


def make_in_maps(prediction, target, consts):
    # fold validity into the prediction: invalid positives -> +50
    # (F^k = e^{-50k} -> 0), invalid negatives -> -50 (E = e^-50 -> 0)
    fill = np.empty((1, N), np.float32)
    fill[:, 0:K] = 50.0
    fill[:, K:N] = -50.0
    pred_m = np.where(target == -1, fill, prediction).astype(np.float32)
    csv = _consts()
    in_maps = []
    for c in range(NC):
        blk = pred_m[c * RPC:(c + 1) * RPC]
        negs = np.ascontiguousarray(blk[:, K:N]).astype(ml_dtypes.bfloat16)
        # doubled positives with the pair-interleave mask folded in:
        # posd[r, 64h:64h+64] = pos row r if h == r%2 else +50
        posd = np.full((RPC, 2, K), 50.0, np.float32)
        rr = np.arange(RPC)
        posd[rr, rr % 2, :] = blk[:, 0:K]
        in_maps.append({
            "negs": negs,
            "posd": posd.reshape(RPC, 2 * K).astype(ml_dtypes.bfloat16),
            "consts": csv,
        })
    return in_maps


def kernel(prediction, target):
    nc, consts = build_program(1)
    in_maps = make_in_maps(prediction, target, consts)
    res = run_bass_kernel_spmd(nc, in_maps, core_ids=list(range(NC)))
    total = sum(float(res.results[c]["y"][0].sum(dtype=np.float64))
                for c in range(NC))
    return np.float32(total / B)


# revision 14
# speedup vs baseline: 1.2291x; 1.0313x over previous
"""BPR pairwise softplus loss on 8 Trainium2 NeuronCores.

loss = mean_b sum_{i<K, j>=K, both valid} softplus(pred[b,j] - pred[b,i])

Strategy (data parallel over batch, 32 rows/core), folding FOUR negatives
per ln via elementary symmetric polynomials:

  prod_{m=1..4} (1 + F*E_m) = 1 + F*c1 + F^2*c2 + F^3*c3 + F^4*c4
  =>  sum_m softplus(n_m - p) = ln(1 + sum_k F^k c_k),  F = exp(-p), E = exp(n)

Pack-free layout: partition = 4*r + k (row-major, power k innermost), so the
8 contraction partitions of row pair t = (2t, 2t+1) are the contiguous range
8t..8t+8 and the matmul reads the power tile P and coefficient tile q
DIRECTLY - no SBUF->SBUF pack DMAs (each DMA hop costs ~2.7us of fixed
latency: 565 SEQ + 625 HWDGE + 650 engine delay + transfer + 900 sem).

  - P_v[4r+k, 64h+p] = F^{k+1}[r, p] masked to half h == r%2 AND to pairs
    with (r//2)%4 == v: each P_v is ONE ScalarE exp with per-partition
    scale -(k+1) / bias 0 on live partitions and scale 0 / bias -100
    (exp -> 0) elsewhere; the column-interleave mask is folded into the
    host data (+50 fill -> exp(-(k+1)*50) = 0).
  - q[4r+k, j] = c_{k+1}[r, j]: pair folds a/m then masked placement with
    per-partition 0/1 scalar masks via scalar_tensor_tensor:
      q = M1*(a1+a2) + M2*(m1+m2+a1*a2) + M3*(a1*m2+a2*m1) + M4*(m1*m2)
  - 16 matmuls (contraction 32 = one aligned block of 4 pairs, lhsT P_v
    zeroing the other 3 pairs, free 112) straight into PSUM; asymmetric Ln
    passes (N0 pairs then 16-N0) with accum_out row sums; one output DMA.
    (PE tiling: operands must sit at partition base 0/32/64/96 with an
    explicit tile_position, hence the 32-block contraction.)
  - Dummy warm matmuls from ~1us keep the PE p-state ramping so the real
    matmuls run at full clock (a PE idle gap resets the 3us ramp).

Invalid slots (target == -1) fold into the prediction on the host: invalid
positives -> +50 (F^k -> 0), invalid negatives -> -50 (E -> 0).
"""
import sys

sys.path.insert(0, "/opt/trn_rl_repo")

import numpy as np
import ml_dtypes

import concourse.bass as bass
import concourse.mybir as mybir
from concourse import bacc
import concourse.hw_specs as hw_specs
from concourse.tile import TileContext
from concourse.bass_utils import run_bass_kernel_spmd

B, N, K = 256, 512, 64
NC = 8
RPC = B // NC            # 32 batch rows per core
NPAIR = RPC // 2         # 16 row pairs (2t, 2t+1)
NEG = N - K              # 448 negatives per row
G = 4                    # negatives folded per ln
NQ = NEG // G            # 112 quad groups per row
NPASS = 2                # Ln passes
N0 = 4                   # row pairs in Ln pass 0 (rest in pass 1); multiple
                         # of 4 so each pass covers whole PSUM banks
N_WARM = 440             # PE clock warm-up dummy matmuls
WARM_COLS = 16           # free size of each warm matmul

_PROG_CACHE = {}

EXP = mybir.ActivationFunctionType.Exp
LN = mybir.ActivationFunctionType.Ln
F32 = mybir.dt.float32
BF16 = mybir.dt.bfloat16


def _patch_act_tables():
    """Make natural_log_exp_and_others the only table set advertising exp/ln
    so Bacc's table-load pass emits a single ACT_TABLE_LOAD."""
    if getattr(hw_specs.get_activation_tables, "_bpr_patched", False):
        return
    orig_fn = hw_specs.get_activation_tables

    def patched(arch):
        d = orig_fn(arch)
        out = {}
        for name, funcs in d.items():
            if name != "natural_log_exp_and_others" and (EXP in funcs
                                                         or LN in funcs):
                funcs = funcs - {EXP, LN}
            out[name] = funcs
        return out

    patched._bpr_patched = True
    hw_specs.get_activation_tables = patched
    bacc.get_activation_tables = patched


def build_program(nreps: int = 1):
    if nreps in _PROG_CACHE:
        return _PROG_CACHE[nreps]
    _patch_act_tables()
    nc = bacc.Bacc("TRN2", target_bir_lowering=False, debug=False,
                   num_devices=NC)
    negs_d = nc.dram_tensor("negs", [RPC, NEG], BF16, kind="ExternalInput")
    # pre-masked doubled positives: [r, 64h+p] = pred_pos[r,p] if h==r%2
    # else +50
    posd_d = nc.dram_tensor("posd", [RPC, 2 * K], BF16, kind="ExternalInput")
    # cols 0-3: P_v exp scales; cols 4-7: P_v exp biases; cols 8-11: one-hot
    # coefficient masks M1..M4 (p%4 == k)
    consts_d = nc.dram_tensor("consts", [128, 16], F32, kind="ExternalInput")
    y = nc.dram_tensor("y", [nreps, 128, NPASS], F32, kind="ExternalOutput")

    mul = mybir.AluOpType.mult
    add = mybir.AluOpType.add

    from contextlib import ExitStack
    with TileContext(nc) as tc, ExitStack() as st:
        io = st.enter_context(tc.tile_pool(name="io", bufs=1))
        ps = st.enter_context(tc.tile_pool(name="ps", bufs=1, space="PSUM"))

        # Trigger the exp/ln activation-table load ASAP (~1.3us on ScalarE,
        # overlapping the input DMA).
        d0 = io.tile([128, 1], F32, tag="d0")
        nc.vector.memset(d0, 0.0)
        d1 = io.tile([128, 1], BF16, tag="d1")
        nc.scalar.activation(d1, d0, EXP)

        # dummy operands for the PE warm-up chain
        dwr = io.tile([8, WARM_COLS], BF16, tag="dwr")
        nc.vector.memset(dwr, 0.0)

        for rep in range(nreps):
            # consts on the Pool SWDGE queue (parallel with SP inputs)
            cs = io.tile([128, 16], F32, tag="cs")
            nc.gpsimd.dma_start(out=cs, in_=consts_d[:])
            m1 = cs[:, 8:9]
            m2 = cs[:, 9:10]
            m3 = cs[:, 10:11]
            m4 = cs[:, 11:12]

            # inputs, replicated 4x across power slots (partition = 4r+k)
            # via stride-0 DRAM reads; negatives first (they gate the chain)
            negs = io.tile([128, NEG], BF16, tag="negs")
            nc.sync.dma_start(
                out=negs,
                in_=negs_d[:].unsqueeze(1).broadcast_to([RPC, 4, NEG]))
            posd = io.tile([128, 2 * K], BF16, tag="posd")
            nc.sync.dma_start(
                out=posd,
                in_=posd_d[:].unsqueeze(1).broadcast_to([RPC, 4, 2 * K]))

            # PE warm-up: keep the clock ramping from ~1us until the real
            # matmuls (psum scratch, no consumers)
            pw = ps.tile([128, WARM_COLS], F32, tag="pw")
            for w in range(N_WARM):
                nc.tensor.matmul(pw[0:WARM_COLS], dwr, dwr,
                                 start=True, stop=True)

            # ---- DVE early window (negs land long before exp finishes):
            # log-space sums so the product folds become ACT exps:
            #   m1 = E1*E2 = exp(n1+n2), m2 = E3*E4 = exp(n3+n4),
            #   v = m1*m2 = exp(n1+n2+n3+n4)
            # f32 sums keep exp() accurate (bf16 sums cost ~3% on exp) ----
            nsum = io.tile([128, 2 * NQ], F32, tag="nsum")
            nsq = io.tile([128, NQ], F32, tag="nsq")
            nc.vector.tensor_tensor(nsum[:, 0:NQ], negs[:, 0:NQ],
                                    negs[:, NQ:2 * NQ], add)
            nc.vector.tensor_tensor(nsum[:, NQ:2 * NQ],
                                    negs[:, 2 * NQ:3 * NQ],
                                    negs[:, 3 * NQ:4 * NQ], add)
            nc.vector.tensor_tensor(nsq, nsum[:, 0:NQ], nsum[:, NQ:2 * NQ],
                                    add)

            # ---- ScalarE stream: exp chunks C1/C2 over negatives, product
            # folds M/V from the log sums, then the four pair-masked power
            # tiles P_v = exp(posd*scale_v + bias_v) ----
            HC = NEG // 2
            e = io.tile([128, NEG], BF16, tag="e")
            nc.scalar.activation(e[:, 0:HC], negs[:, 0:HC], EXP)
            nc.scalar.activation(e[:, HC:NEG], negs[:, HC:NEG], EXP)
            m = io.tile([128, 2 * NQ], BF16, tag="m")
            nc.scalar.activation(m, nsum, EXP)
            v = io.tile([128, NQ], BF16, tag="v")
            nc.scalar.activation(v, nsq, EXP)
            pv = []
            for vi in range(4):
                pvt = io.tile([128, 2 * K], BF16, tag=f"P{vi}")
                nc.scalar.activation(pvt, posd, EXP, scale=cs[:, vi:vi + 1],
                                     bias=cs[:, 4 + vi:5 + vi])
                pv.append(pvt)

            # ---- DVE: additive pair folds ----
            a = io.tile([128, 2 * NQ], BF16, tag="a")
            nc.vector.tensor_tensor(a[:, 0:NQ], e[:, 0:NQ], e[:, NQ:2 * NQ],
                                    add)
            nc.vector.tensor_tensor(a[:, NQ:2 * NQ], e[:, 2 * NQ:3 * NQ],
                                    e[:, 3 * NQ:4 * NQ], add)
            a1 = a[:, 0:NQ]
            a2 = a[:, NQ:2 * NQ]
            mm1 = m[:, 0:NQ]
            mm2 = m[:, NQ:2 * NQ]

            # ---- masked coefficient placement:
            # q[4r+k] = c_{k+1}[r]; Mk are per-partition 0/1 scalars ----
            w = io.tile([128, NQ], BF16, tag="w")
            # one product on Pool (off the DVE critical path); Pool only
            # supports plain tensor_tensor, masking happens in the DVE stt
            # chain below
            nc.gpsimd.tensor_tensor(w, a2, mm1, mul)

            s_a = io.tile([128, NQ], BF16, tag="sa")
            t_aa2 = io.tile([128, NQ], BF16, tag="taa2")
            s_m = io.tile([128, NQ], BF16, tag="sm")
            t_am3 = io.tile([128, NQ], BF16, tag="tam3")
            t_v4 = io.tile([128, NQ], BF16, tag="tv4")
            y1 = io.tile([128, NQ], BF16, tag="y1")
            y2 = io.tile([128, NQ], BF16, tag="y2")
            y3 = io.tile([128, NQ], BF16, tag="y3")
            z = io.tile([128, NQ], BF16, tag="z")
            q = io.tile([128, NQ], BF16, tag="q")
            nc.vector.tensor_tensor(s_a, a1, a2, add)
            nc.vector.scalar_tensor_tensor(t_aa2, a1, m2, a2, mul, mul)
            nc.vector.tensor_tensor(s_m, mm1, mm2, add)
            nc.vector.scalar_tensor_tensor(t_am3, mm2, m3, a1, mul, mul)
            nc.vector.tensor_scalar(t_v4, v, m4, None, mul)
            nc.vector.scalar_tensor_tensor(y1, s_a, m1, t_aa2, mul, add)
            nc.vector.scalar_tensor_tensor(y2, s_m, m2, t_am3, mul, add)
            # z is independent of y1/y2 so the tail has a single
            # engine-completion wait (q on y3) instead of two
            nc.vector.scalar_tensor_tensor(z, w, m3, t_v4, mul, add)
            nc.vector.tensor_tensor(y3, y1, y2, add)
            nc.vector.tensor_tensor(q, y3, z, add)

            # ---- one matmul per row pair t: psum[64h+p, j] =
            # sum_k F^{k+1}[2t+h, p] * c_{k+1}[2t+h, j]; contraction is the
            # aligned 32-block of 4 pairs, P_{t%4} zeroes the other 3 ----
            # psum: 4 pairs per 512-col bank (a matmul output cannot cross a
            # bank boundary); pass 0 = bank 0, pass 1 = banks 1..3
            NB1 = (NPAIR - N0) // 4
            pt0 = ps.tile([128, 512], F32, tag="ps0")
            pt1 = ps.tile([128, NB1 * 512], F32, tag="ps1")
            for t in range(NPAIR):
                if t < N0:
                    out_sl = pt0[:, NQ * t:NQ * (t + 1)]
                else:
                    u = t - N0
                    base = 512 * (u // 4) + NQ * (u % 4)
                    out_sl = pt1[:, base:base + NQ]
                blk = 32 * (t // 4)
                nc.tensor.matmul(out_sl,
                                 pv[t % 4][blk:blk + 32, :],
                                 q[blk:blk + 32, :],
                                 start=True, stop=True,
                                 tile_position=(blk, 0))

            # ---- ln(1 + psum), accumulated per partition; asymmetric
            # passes so pass 0 starts after only N0 pairs of matmuls.
            # Throwaway ln outputs land in spare PSUM (access init 172 cyc
            # < SBUF 222). ----
            partials = io.tile([128, NPASS], F32, tag="part")
            sout = ps.tile([128, (NPAIR - N0) * NQ], F32, tag="souts")
            nc.scalar.activation(sout[:, 0:N0 * NQ], pt0[:, 0:N0 * NQ], LN,
                                 bias=1.0, accum_out=partials[:, 0:1])
            nc.scalar.activation(
                sout[:, 0:(NPAIR - N0) * NQ].rearrange(
                    "p (b x) -> p b x", x=4 * NQ),
                pt1.rearrange("p (b x) -> p b x", x=512)[:, :, 0:4 * NQ],
                LN, bias=1.0, accum_out=partials[:, 1:2])

            nc.sync.dma_start(out=y[rep], in_=partials)

    nc.finalize()
    _PROG_CACHE[nreps] = (nc, ())
    return nc, ()


def _consts():
    cs = np.zeros((128, 16), dtype=np.float32)
    p = np.arange(128)
    k = p % 4
    vblk = (p % 32) // 8  # which pair-of-the-32-block this partition feeds
    for v in range(4):
        live = vblk == v
        cs[:, v] = np.where(live, -(1.0 + k), 0.0)
        cs[:, 4 + v] = np.where(live, 0.0, -100.0)
    for i in range(4):
        cs[:, 8 + i] = (k == i).astype(np.float32)
    return cs


def make_in_maps(prediction, target, consts):
    # fold validity into the prediction: invalid positives -> +50
    # (F^k = e^{-50k} -> 0), invalid negatives -> -50 (E = e^-50 -> 0)
    fill = np.empty((1, N), np.float32)
    fill[:, 0:K] = 50.0
    fill[:, K:N] = -50.0
    pred_m = np.where(target == -1, fill, prediction).astype(np.float32)
    csv = _consts()
    in_maps = []
    for c in range(NC):
        blk = pred_m[c * RPC:(c + 1) * RPC]
        negs = np.ascontiguousarray(blk[:, K:N]).astype(ml_dtypes.bfloat16)
        # doubled positives with the pair-interleave mask folded in:
        # posd[r, 64h:64h+64] = pos row r if h == r%2 else +50
        posd = np.full((RPC, 2, K), 50.0, np.float32)
        rr = np.arange(RPC)
        posd[rr, rr % 2, :] = blk[:, 0:K]
        in_maps.append({
            "negs": negs,
            "posd": posd.reshape(RPC, 2 * K).astype(ml_dtypes.bfloat16),
            "consts": csv,
        })
    return in_maps


def kernel(prediction, target):
    nc, consts = build_program(1)
    in_maps = make_in_maps(prediction, target, consts)
    res = run_bass_kernel_spmd(nc, in_maps, core_ids=list(range(NC)))
    total = sum(float(res.results[c]["y"][0].sum(dtype=np.float64))
                for c in range(NC))
    return np.float32(total / B)


# revision 23
# speedup vs baseline: 1.2300x; 1.0007x over previous
"""BPR pairwise softplus loss on 8 Trainium2 NeuronCores.

loss = mean_b sum_{i<K, j>=K, both valid} softplus(pred[b,j] - pred[b,i])

Strategy (data parallel over batch, 32 rows/core), folding FOUR negatives
per ln via elementary symmetric polynomials:

  prod_{m=1..4} (1 + F*E_m) = 1 + F*c1 + F^2*c2 + F^3*c3 + F^4*c4
  =>  sum_m softplus(n_m - p) = ln(1 + sum_k F^k c_k),  F = exp(-p), E = exp(n)

Pack-free layout: partition = 4*r + k (row-major, power k innermost), so the
8 contraction partitions of row pair t = (2t, 2t+1) are the contiguous range
8t..8t+8 and the matmul reads the power tile P and coefficient tile q
DIRECTLY - no SBUF->SBUF pack DMAs (each DMA hop costs ~2.7us of fixed
latency: 565 SEQ + 625 HWDGE + 650 engine delay + transfer + 900 sem).

  - P_v[4r+k, 64h+p] = F^{k+1}[r, p] masked to half h == r%2 AND to pairs
    with (r//2)%4 == v: each P_v is ONE ScalarE exp with per-partition
    scale -(k+1) / bias 0 on live partitions and scale 0 / bias -100
    (exp -> 0) elsewhere; the column-interleave mask is folded into the
    host data (+50 fill -> exp(-(k+1)*50) = 0).
  - q[4r+k, j] = c_{k+1}[r, j]: pair folds a/m then masked placement with
    per-partition 0/1 scalar masks via scalar_tensor_tensor:
      q = M1*(a1+a2) + M2*(m1+m2+a1*a2) + M3*(a1*m2+a2*m1) + M4*(m1*m2)
  - 16 matmuls (contraction 32 = one aligned block of 4 pairs, lhsT P_v
    zeroing the other 3 pairs, free 112) straight into PSUM; asymmetric Ln
    passes (N0 pairs then 16-N0) with accum_out row sums; one output DMA.
    (PE tiling: operands must sit at partition base 0/32/64/96 with an
    explicit tile_position, hence the 32-block contraction.)
  - Dummy warm matmuls from ~1us keep the PE p-state ramping so the real
    matmuls run at full clock (a PE idle gap resets the 3us ramp).

Invalid slots (target == -1) fold into the prediction on the host: invalid
positives -> +50 (F^k -> 0), invalid negatives -> -50 (E -> 0).
"""
import sys

sys.path.insert(0, "/opt/trn_rl_repo")

import numpy as np
import ml_dtypes

import concourse.bass as bass
import concourse.mybir as mybir
from concourse import bacc
import concourse.hw_specs as hw_specs
from concourse.tile import TileContext
from concourse.bass_utils import run_bass_kernel_spmd

B, N, K = 256, 512, 64
NC = 8
RPC = B // NC            # 32 batch rows per core
NPAIR = RPC // 2         # 16 row pairs (2t, 2t+1)
NEG = N - K              # 448 negatives per row
G = 4                    # negatives folded per ln
NQ = NEG // G            # 112 quad groups per row
NPASS = 2                # Ln passes
N0 = 4                   # row pairs in Ln pass 0 (rest in pass 1); multiple
                         # of 4 so each pass covers whole PSUM banks
N_WARM = 440             # PE clock warm-up dummy matmuls
WARM_COLS = 16           # free size of each warm matmul

_PROG_CACHE = {}

EXP = mybir.ActivationFunctionType.Exp
LN = mybir.ActivationFunctionType.Ln
F32 = mybir.dt.float32
BF16 = mybir.dt.bfloat16


def _patch_act_tables():
    """Make natural_log_exp_and_others the only table set advertising exp/ln
    so Bacc's table-load pass emits a single ACT_TABLE_LOAD."""
    if getattr(hw_specs.get_activation_tables, "_bpr_patched", False):
        return
    orig_fn = hw_specs.get_activation_tables

    def patched(arch):
        d = orig_fn(arch)
        out = {}
        for name, funcs in d.items():
            if name != "natural_log_exp_and_others" and (EXP in funcs
                                                         or LN in funcs):
                funcs = funcs - {EXP, LN}
            out[name] = funcs
        return out

    patched._bpr_patched = True
    hw_specs.get_activation_tables = patched
    bacc.get_activation_tables = patched


def build_program(nreps: int = 1):
    if nreps in _PROG_CACHE:
        return _PROG_CACHE[nreps]
    _patch_act_tables()
    nc = bacc.Bacc("TRN2", target_bir_lowering=False, debug=False,
                   num_devices=NC)
    negs_d = nc.dram_tensor("negs", [RPC, NEG], BF16, kind="ExternalInput")
    # pre-masked doubled positives, replicated 4x over the pair-of-block
    # index v with the v-mask baked in on the host:
    # posd4[r, v, 64h+p] = pred_pos[r,p] if (h==r%2 and (r%8)//2==v) else +50
    posd_d = nc.dram_tensor("posd", [RPC, 4 * 2 * K], BF16,
                            kind="ExternalInput")
    # col 0: P exp scale -(1+p%4); cols 8-11: one-hot coefficient masks
    # M1..M4 (p%4 == k)
    consts_d = nc.dram_tensor("consts", [128, 16], F32, kind="ExternalInput")
    y = nc.dram_tensor("y", [nreps, 128, NPASS], F32, kind="ExternalOutput")

    mul = mybir.AluOpType.mult
    add = mybir.AluOpType.add

    from contextlib import ExitStack
    with TileContext(nc) as tc, ExitStack() as st:
        io = st.enter_context(tc.tile_pool(name="io", bufs=1))
        ps = st.enter_context(tc.tile_pool(name="ps", bufs=1, space="PSUM"))

        # Trigger the exp/ln activation-table load ASAP (~1.3us on ScalarE,
        # overlapping the input DMA).
        d0 = io.tile([128, 1], F32, tag="d0")
        nc.vector.memset(d0, 0.0)
        d1 = io.tile([128, 1], BF16, tag="d1")
        nc.scalar.activation(d1, d0, EXP)

        # dummy operands for the PE warm-up chain
        dwr = io.tile([8, WARM_COLS], BF16, tag="dwr")
        nc.vector.memset(dwr, 0.0)

        for rep in range(nreps):
            # consts on the Pool SWDGE queue (parallel with SP inputs)
            cs = io.tile([128, 16], F32, tag="cs")
            nc.gpsimd.dma_start(out=cs, in_=consts_d[:])
            m1 = cs[:, 8:9]
            m2 = cs[:, 9:10]
            m3 = cs[:, 10:11]
            m4 = cs[:, 11:12]

            # inputs, replicated 4x across power slots (partition = 4r+k)
            # via stride-0 DRAM reads; negatives first (they gate the chain)
            negs = io.tile([128, NEG], BF16, tag="negs")
            nc.sync.dma_start(
                out=negs,
                in_=negs_d[:].unsqueeze(1).broadcast_to([RPC, 4, NEG]))
            posd = io.tile([128, 4 * 2 * K], BF16, tag="posd")
            nc.sync.dma_start(
                out=posd,
                in_=posd_d[:].unsqueeze(1).broadcast_to([RPC, 4, 4 * 2 * K]))

            # PE warm-up: keep the clock ramping from ~1us until the real
            # matmuls (psum scratch, no consumers)
            pw = ps.tile([128, WARM_COLS], F32, tag="pw")
            for w in range(N_WARM):
                nc.tensor.matmul(pw[0:WARM_COLS], dwr, dwr,
                                 start=True, stop=True)

            # ---- DVE early window (negs land long before exp finishes):
            # log-space sums so the product folds become ACT exps:
            #   m1 = E1*E2 = exp(n1+n2), m2 = E3*E4 = exp(n3+n4),
            #   v = m1*m2 = exp(n1+n2+n3+n4)
            # f32 sums keep exp() accurate (bf16 sums cost ~3% on exp) ----
            nsum = io.tile([128, 2 * NQ], F32, tag="nsum")
            nc.vector.tensor_tensor(nsum[:, 0:NQ], negs[:, 0:NQ],
                                    negs[:, NQ:2 * NQ], add)
            nc.vector.tensor_tensor(nsum[:, NQ:2 * NQ],
                                    negs[:, 2 * NQ:3 * NQ],
                                    negs[:, 3 * NQ:4 * NQ], add)

            # ---- ScalarE stream: exp chunks C1/C2 over negatives, product
            # folds M/V from the log sums, then the four pair-masked power
            # tiles P_v = exp(posd*scale_v + bias_v) ----
            HC = NEG // 2
            e = io.tile([128, NEG], BF16, tag="e")
            nc.scalar.activation(e[:, 0:HC], negs[:, 0:HC], EXP)
            nc.scalar.activation(e[:, HC:NEG], negs[:, HC:NEG], EXP)
            m = io.tile([128, 2 * NQ], BF16, tag="m")
            nc.scalar.activation(m, nsum, EXP)
            # ONE exp for all four pair-masked power tiles (v-mask baked in
            # the host data, power k in the per-partition scale): P_all's
            # column block v holds P_v
            p_all = io.tile([128, 4 * 2 * K], BF16, tag="Pall")
            nc.scalar.activation(p_all, posd, EXP, scale=cs[:, 0:1])
            pv = [p_all[:, 2 * K * vi:2 * K * (vi + 1)] for vi in range(4)]

            # ---- DVE: additive pair folds ----
            a = io.tile([128, 2 * NQ], BF16, tag="a")
            nc.vector.tensor_tensor(a[:, 0:NQ], e[:, 0:NQ], e[:, NQ:2 * NQ],
                                    add)
            nc.vector.tensor_tensor(a[:, NQ:2 * NQ], e[:, 2 * NQ:3 * NQ],
                                    e[:, 3 * NQ:4 * NQ], add)
            a1 = a[:, 0:NQ]
            a2 = a[:, NQ:2 * NQ]
            mm1 = m[:, 0:NQ]
            mm2 = m[:, NQ:2 * NQ]

            # ---- masked coefficient placement:
            # q[4r+k] = c_{k+1}[r]; Mk are per-partition 0/1 scalars ----
            w = io.tile([128, NQ], BF16, tag="w")
            v = io.tile([128, NQ], BF16, tag="v")
            # two products on Pool (off the DVE critical path); Pool only
            # supports plain tensor_tensor, masking happens in the DVE stt
            # chain below
            nc.gpsimd.tensor_tensor(w, a2, mm1, mul)
            nc.gpsimd.tensor_tensor(v, mm1, mm2, mul)

            s_a = io.tile([128, NQ], BF16, tag="sa")
            t_aa2 = io.tile([128, NQ], BF16, tag="taa2")
            s_m = io.tile([128, NQ], BF16, tag="sm")
            t_am3 = io.tile([128, NQ], BF16, tag="tam3")
            t_v4 = io.tile([128, NQ], BF16, tag="tv4")
            y1 = io.tile([128, NQ], BF16, tag="y1")
            y2 = io.tile([128, NQ], BF16, tag="y2")
            y3 = io.tile([128, NQ], BF16, tag="y3")
            z = io.tile([128, NQ], BF16, tag="z")
            q = io.tile([128, NQ], BF16, tag="q")
            nc.vector.tensor_tensor(s_a, a1, a2, add)
            nc.vector.scalar_tensor_tensor(t_aa2, a1, m2, a2, mul, mul)
            nc.vector.tensor_tensor(s_m, mm1, mm2, add)
            nc.vector.scalar_tensor_tensor(t_am3, mm2, m3, a1, mul, mul)
            nc.vector.tensor_scalar(t_v4, v, m4, None, mul)
            nc.vector.scalar_tensor_tensor(y1, s_a, m1, t_aa2, mul, add)
            nc.vector.scalar_tensor_tensor(y2, s_m, m2, t_am3, mul, add)
            # z is independent of y1/y2 so the tail has a single
            # engine-completion wait (q on y3) instead of two
            nc.vector.scalar_tensor_tensor(z, w, m3, t_v4, mul, add)
            nc.vector.tensor_tensor(y3, y1, y2, add)
            nc.vector.tensor_tensor(q, y3, z, add)

            # ---- matmuls: psum[64h+p, j] = sum_k F^{k+1}[2t+h, p] *
            # c_{k+1}[2t+h, j]; contraction is the aligned 32-block of 4
            # pairs, P_{t%4} zeroes the other 3.  psum: 4 pairs per 512-col
            # bank (a matmul output cannot cross a bank boundary).
            #
            # Matmul is linear in rhs, so pass-0 pairs {0,4,8,12} (all v=0,
            # needing only P0 which lands first) accumulate THREE early
            # matmul waves rhs = y1, y2, z — their PSUM is complete before
            # the q tile even exists, pulling the first Ln pass forward.
            # Remaining pairs use a single matmul on q = y3 + z. ----
            NB1 = (NPAIR - N0) // 4
            # each pass-0 pair gets its OWN bank (own 2KB zero region), so
            # the three accumulation waves form independent per-bank groups
            pt0 = ps.tile([128, N0 * 512], F32, tag="ps0")
            pt1 = ps.tile([128, NB1 * 512], F32, tag="ps1")
            p0_pairs = [0, 1, 2, 3]
            for wi, rhs_t in enumerate((y1, y2, z)):
                for i, t in enumerate(p0_pairs):
                    blk = 32 * (t // 4)
                    nc.tensor.matmul(pt0[:, 512 * i:512 * i + NQ],
                                     pv[t % 4][blk:blk + 32, :],
                                     rhs_t[blk:blk + 32, :],
                                     start=(wi == 0), stop=(wi == 2),
                                     tile_position=(blk, 0))
            u = 0
            for t in range(NPAIR):
                if t in p0_pairs:
                    continue
                base = 512 * (u // 4) + NQ * (u % 4)
                blk = 32 * (t // 4)
                nc.tensor.matmul(pt1[:, base:base + NQ],
                                 pv[t % 4][blk:blk + 32, :],
                                 q[blk:blk + 32, :],
                                 start=True, stop=True,
                                 tile_position=(blk, 0))
                u += 1

            # ---- ln(1 + psum), accumulated per partition; asymmetric
            # passes so pass 0 starts after only N0 pairs of matmuls.
            # Throwaway ln outputs land in spare PSUM (access init 172 cyc
            # < SBUF 222). ----
            # Ln pass 0's throwaway output goes to SBUF; pass 1's overwrites
            # the then-dead pt0 banks (PSUM budget: N0+NB1 banks + warm = 8)
            partials = io.tile([128, NPASS], F32, tag="part")
            sout0 = io.tile([128, N0 * NQ], BF16, tag="sout0")
            nc.scalar.activation(
                sout0.rearrange("p (b x) -> p b x", x=NQ),
                pt0.rearrange("p (b x) -> p b x", x=512)[:, :, 0:NQ],
                LN, bias=1.0, accum_out=partials[:, 0:1])
            nc.scalar.activation(
                pt0[:, 0:(NPAIR - N0) * NQ].rearrange(
                    "p (b x) -> p b x", x=4 * NQ),
                pt1.rearrange("p (b x) -> p b x", x=512)[:, :, 0:4 * NQ],
                LN, bias=1.0, accum_out=partials[:, 1:2])

            nc.sync.dma_start(out=y[rep], in_=partials)

    nc.finalize()
    _PROG_CACHE[nreps] = (nc, ())
    return nc, ()


def _consts():
    cs = np.zeros((128, 16), dtype=np.float32)
    p = np.arange(128)
    k = p % 4
    cs[:, 0] = -(1.0 + k)
    for i in range(4):
        cs[:, 8 + i] = (k == i).astype(np.float32)
    return cs


def make_in_maps(prediction, target, consts):
    # fold validity into the prediction: invalid positives -> +50
    # (F^k = e^{-50k} -> 0), invalid negatives -> -50 (E = e^-50 -> 0)
    fill = np.empty((1, N), np.float32)
    fill[:, 0:K] = 50.0
    fill[:, K:N] = -50.0
    pred_m = np.where(target == -1, fill, prediction).astype(np.float32)
    csv = _consts()
    in_maps = []
    for c in range(NC):
        blk = pred_m[c * RPC:(c + 1) * RPC]
        negs = np.ascontiguousarray(blk[:, K:N]).astype(ml_dtypes.bfloat16)
        # doubled positives with the pair-interleave AND pair-of-block
        # masks folded in: posd4[r, v, 64h:64h+64] = pos row r if
        # (h == r%2 and (r%8)//2 == v) else +50
        posd4 = np.full((RPC, 4, 2, K), 50.0, np.float32)
        rr = np.arange(RPC)
        posd4[rr, (rr % 8) // 2, rr % 2, :] = blk[:, 0:K]
        in_maps.append({
            "negs": negs,
            "posd": posd4.reshape(RPC, 4 * 2 * K).astype(ml_dtypes.bfloat16),
            "consts": csv,
        })
    return in_maps


def kernel(prediction, target):
    nc, consts = build_program(1)
    in_maps = make_in_maps(prediction, target, consts)
    res = run_bass_kernel_spmd(nc, in_maps, core_ids=list(range(NC)))
    total = sum(float(res.results[c]["y"][0].sum(dtype=np.float64))
                for c in range(NC))
    return np.float32(total / B)


# revision 24
# speedup vs baseline: 1.2398x; 1.0080x over previous
"""BPR pairwise softplus loss on 8 Trainium2 NeuronCores.

loss = mean_b sum_{i<K, j>=K, both valid} softplus(pred[b,j] - pred[b,i])

Strategy (data parallel over batch, 32 rows/core), folding FOUR negatives
per ln via elementary symmetric polynomials:

  prod_{m=1..4} (1 + F*E_m) = 1 + F*c1 + F^2*c2 + F^3*c3 + F^4*c4
  =>  sum_m softplus(n_m - p) = ln(1 + sum_k F^k c_k),  F = exp(-p), E = exp(n)

Pack-free layout: partition = 4*r + k (row-major, power k innermost), so the
8 contraction partitions of row pair t = (2t, 2t+1) are the contiguous range
8t..8t+8 and the matmul reads the power tile P and coefficient tile q
DIRECTLY - no SBUF->SBUF pack DMAs (each DMA hop costs ~2.7us of fixed
latency: 565 SEQ + 625 HWDGE + 650 engine delay + transfer + 900 sem).

  - P_v[4r+k, 64h+p] = F^{k+1}[r, p] masked to half h == r%2 AND to pairs
    with (r//2)%4 == v: each P_v is ONE ScalarE exp with per-partition
    scale -(k+1) / bias 0 on live partitions and scale 0 / bias -100
    (exp -> 0) elsewhere; the column-interleave mask is folded into the
    host data (+50 fill -> exp(-(k+1)*50) = 0).
  - q[4r+k, j] = c_{k+1}[r, j]: pair folds a/m then masked placement with
    per-partition 0/1 scalar masks via scalar_tensor_tensor:
      q = M1*(a1+a2) + M2*(m1+m2+a1*a2) + M3*(a1*m2+a2*m1) + M4*(m1*m2)
  - 16 matmuls (contraction 32 = one aligned block of 4 pairs, lhsT P_v
    zeroing the other 3 pairs, free 112) straight into PSUM; asymmetric Ln
    passes (N0 pairs then 16-N0) with accum_out row sums; one output DMA.
    (PE tiling: operands must sit at partition base 0/32/64/96 with an
    explicit tile_position, hence the 32-block contraction.)
  - Dummy warm matmuls from ~1us keep the PE p-state ramping so the real
    matmuls run at full clock (a PE idle gap resets the 3us ramp).

Invalid slots (target == -1) fold into the prediction on the host: invalid
positives -> +50 (F^k -> 0), invalid negatives -> -50 (E -> 0).
"""
import sys

sys.path.insert(0, "/opt/trn_rl_repo")

import numpy as np
import ml_dtypes

import concourse.bass as bass
import concourse.mybir as mybir
from concourse import bacc
import concourse.hw_specs as hw_specs
from concourse.tile import TileContext
from concourse.bass_utils import run_bass_kernel_spmd

B, N, K = 256, 512, 64
NC = 8
RPC = B // NC            # 32 batch rows per core
NPAIR = RPC // 2         # 16 row pairs (2t, 2t+1)
NEG = N - K              # 448 negatives per row
G = 4                    # negatives folded per ln
NQ = NEG // G            # 112 quad groups per row
NPASS = 2                # Ln passes
N0 = 4                   # row pairs in Ln pass 0 (rest in pass 1); multiple
                         # of 4 so each pass covers whole PSUM banks
N_WARM = 440             # PE clock warm-up dummy matmuls
WARM_COLS = 16           # free size of each warm matmul

_PROG_CACHE = {}

EXP = mybir.ActivationFunctionType.Exp
LN = mybir.ActivationFunctionType.Ln
F32 = mybir.dt.float32
BF16 = mybir.dt.bfloat16


def _patch_act_tables():
    """Make natural_log_exp_and_others the only table set advertising exp/ln
    so Bacc's table-load pass emits a single ACT_TABLE_LOAD."""
    if getattr(hw_specs.get_activation_tables, "_bpr_patched", False):
        return
    orig_fn = hw_specs.get_activation_tables

    def patched(arch):
        d = orig_fn(arch)
        out = {}
        for name, funcs in d.items():
            if name != "natural_log_exp_and_others" and (EXP in funcs
                                                         or LN in funcs):
                funcs = funcs - {EXP, LN}
            out[name] = funcs
        return out

    patched._bpr_patched = True
    hw_specs.get_activation_tables = patched
    bacc.get_activation_tables = patched


def build_program(nreps: int = 1):
    if nreps in _PROG_CACHE:
        return _PROG_CACHE[nreps]
    _patch_act_tables()
    nc = bacc.Bacc("TRN2", target_bir_lowering=False, debug=False,
                   num_devices=NC)
    negs_d = nc.dram_tensor("negs", [RPC, NEG], BF16, kind="ExternalInput")
    # pre-masked doubled positives, replicated 4x over the pair-of-block
    # index v with the v-mask baked in on the host:
    # posd4[r, v, 64h+p] = pred_pos[r,p] if (h==r%2 and (r%8)//2==v) else +50
    posd_d = nc.dram_tensor("posd", [RPC, 4 * 2 * K], BF16,
                            kind="ExternalInput")
    # col 0: P exp scale -(1+p%4); cols 8-11: one-hot coefficient masks
    # M1..M4 (p%4 == k)
    consts_d = nc.dram_tensor("consts", [128, 16], F32, kind="ExternalInput")
    y = nc.dram_tensor("y", [nreps, 128, NPASS], F32, kind="ExternalOutput")

    mul = mybir.AluOpType.mult
    add = mybir.AluOpType.add

    from contextlib import ExitStack
    with TileContext(nc) as tc, ExitStack() as st:
        io = st.enter_context(tc.tile_pool(name="io", bufs=1))
        ps = st.enter_context(tc.tile_pool(name="ps", bufs=1, space="PSUM"))

        # Trigger the exp/ln activation-table load ASAP (~1.3us on ScalarE,
        # overlapping the input DMA).
        d0 = io.tile([128, 1], F32, tag="d0")
        nc.vector.memset(d0, 0.0)
        d1 = io.tile([128, 1], BF16, tag="d1")
        nc.scalar.activation(d1, d0, EXP)

        # dummy operands for the PE warm-up chain
        dwr = io.tile([8, WARM_COLS], BF16, tag="dwr")
        nc.vector.memset(dwr, 0.0)

        for rep in range(nreps):
            # consts on the Pool SWDGE queue (parallel with SP inputs)
            cs = io.tile([128, 16], F32, tag="cs")
            nc.gpsimd.dma_start(out=cs, in_=consts_d[:])
            m1 = cs[:, 8:9]
            m2 = cs[:, 9:10]
            m3 = cs[:, 10:11]
            m4 = cs[:, 11:12]

            # inputs, replicated 4x across power slots (partition = 4r+k)
            # via stride-0 DRAM reads; negatives first (they gate the chain)
            negs = io.tile([128, NEG], BF16, tag="negs")
            nc.sync.dma_start(
                out=negs,
                in_=negs_d[:].unsqueeze(1).broadcast_to([RPC, 4, NEG]))
            posd = io.tile([128, 4 * 2 * K], BF16, tag="posd")
            nc.sync.dma_start(
                out=posd,
                in_=posd_d[:].unsqueeze(1).broadcast_to([RPC, 4, 4 * 2 * K]))

            # PE warm-up: keep the clock ramping from ~1us until the real
            # matmuls (psum scratch, no consumers)
            pw = ps.tile([128, WARM_COLS], F32, tag="pw")
            for w in range(N_WARM):
                nc.tensor.matmul(pw[0:WARM_COLS], dwr, dwr,
                                 start=True, stop=True)

            # ---- DVE early window (negs land long before exp finishes):
            # log-space sums so the product folds become ACT exps:
            #   m1 = E1*E2 = exp(n1+n2), m2 = E3*E4 = exp(n3+n4),
            #   v = m1*m2 = exp(n1+n2+n3+n4)
            # f32 sums keep exp() accurate (bf16 sums cost ~3% on exp) ----
            nsum = io.tile([128, 2 * NQ], F32, tag="nsum")
            nc.vector.tensor_tensor(nsum[:, 0:NQ], negs[:, 0:NQ],
                                    negs[:, NQ:2 * NQ], add)
            nc.vector.tensor_tensor(nsum[:, NQ:2 * NQ],
                                    negs[:, 2 * NQ:3 * NQ],
                                    negs[:, 3 * NQ:4 * NQ], add)

            # ---- ScalarE stream: exp chunks C1/C2 over negatives, product
            # folds M/V from the log sums, then the four pair-masked power
            # tiles P_v = exp(posd*scale_v + bias_v) ----
            HC = NEG // 2
            e = io.tile([128, NEG], BF16, tag="e")
            nc.scalar.activation(e[:, 0:HC], negs[:, 0:HC], EXP)
            nc.scalar.activation(e[:, HC:NEG], negs[:, HC:NEG], EXP)
            m = io.tile([128, 2 * NQ], BF16, tag="m")
            nc.scalar.activation(m, nsum, EXP)
            # ONE exp for all four pair-masked power tiles (v-mask baked in
            # the host data, power k in the per-partition scale): P_all's
            # column block v holds P_v
            p_all = io.tile([128, 4 * 2 * K], BF16, tag="Pall")
            nc.scalar.activation(p_all, posd, EXP, scale=cs[:, 0:1])
            pv = [p_all[:, 2 * K * vi:2 * K * (vi + 1)] for vi in range(4)]

            # ---- DVE: additive pair folds ----
            a = io.tile([128, 2 * NQ], BF16, tag="a")
            nc.vector.tensor_tensor(a[:, 0:NQ], e[:, 0:NQ], e[:, NQ:2 * NQ],
                                    add)
            nc.vector.tensor_tensor(a[:, NQ:2 * NQ], e[:, 2 * NQ:3 * NQ],
                                    e[:, 3 * NQ:4 * NQ], add)
            a1 = a[:, 0:NQ]
            a2 = a[:, NQ:2 * NQ]
            mm1 = m[:, 0:NQ]
            mm2 = m[:, NQ:2 * NQ]

            # ---- masked coefficient placement:
            # q[4r+k] = c_{k+1}[r]; Mk are per-partition 0/1 scalars ----
            w = io.tile([128, NQ], BF16, tag="w")
            v = io.tile([128, NQ], BF16, tag="v")
            # two products on Pool (off the DVE critical path); Pool only
            # supports plain tensor_tensor, masking happens in the DVE stt
            # chain below
            nc.gpsimd.tensor_tensor(v, mm1, mm2, mul)
            nc.gpsimd.tensor_tensor(w, a2, mm1, mul)

            s_a = io.tile([128, NQ], BF16, tag="sa")
            t_aa2 = io.tile([128, NQ], BF16, tag="taa2")
            s_m = io.tile([128, NQ], BF16, tag="sm")
            t_am3 = io.tile([128, NQ], BF16, tag="tam3")
            t_v4 = io.tile([128, NQ], BF16, tag="tv4")
            y1 = io.tile([128, NQ], BF16, tag="y1")
            y2 = io.tile([128, NQ], BF16, tag="y2")
            y3 = io.tile([128, NQ], BF16, tag="y3")
            z = io.tile([128, NQ], BF16, tag="z")
            q = io.tile([128, NQ], BF16, tag="q")
            nc.vector.tensor_tensor(s_a, a1, a2, add)
            nc.vector.scalar_tensor_tensor(t_aa2, a1, m2, a2, mul, mul)
            nc.vector.tensor_tensor(s_m, mm1, mm2, add)
            nc.vector.scalar_tensor_tensor(t_am3, mm2, m3, a1, mul, mul)
            nc.vector.scalar_tensor_tensor(y1, s_a, m1, t_aa2, mul, add)
            nc.vector.scalar_tensor_tensor(y2, s_m, m2, t_am3, mul, add)
            # t_v4/z issue after y1/y2 so the in-order queue does not park
            # the chain on Pool's v; z is independent of y1/y2 so the tail
            # has a single engine-completion wait (q on y3)
            nc.vector.tensor_scalar(t_v4, v, m4, None, mul)
            nc.vector.scalar_tensor_tensor(z, w, m3, t_v4, mul, add)
            nc.vector.tensor_tensor(y3, y1, y2, add)
            nc.vector.tensor_tensor(q, y3, z, add)

            # ---- matmuls: psum[64h+p, j] = sum_k F^{k+1}[2t+h, p] *
            # c_{k+1}[2t+h, j]; contraction is the aligned 32-block of 4
            # pairs, P_{t%4} zeroes the other 3.  psum: 4 pairs per 512-col
            # bank (a matmul output cannot cross a bank boundary).
            #
            # Matmul is linear in rhs, so pass-0 pairs {0,4,8,12} (all v=0,
            # needing only P0 which lands first) accumulate THREE early
            # matmul waves rhs = y1, y2, z — their PSUM is complete before
            # the q tile even exists, pulling the first Ln pass forward.
            # Remaining pairs use a single matmul on q = y3 + z. ----
            NB1 = (NPAIR - N0) // 4
            # each pass-0 pair gets its OWN bank (own 2KB zero region), so
            # the three accumulation waves form independent per-bank groups
            pt0 = ps.tile([128, N0 * 512], F32, tag="ps0")
            pt1 = ps.tile([128, NB1 * 512], F32, tag="ps1")
            p0_pairs = [0, 1, 2, 3]
            for wi, rhs_t in enumerate((y1, y2, z)):
                for i, t in enumerate(p0_pairs):
                    blk = 32 * (t // 4)
                    nc.tensor.matmul(pt0[:, 512 * i:512 * i + NQ],
                                     pv[t % 4][blk:blk + 32, :],
                                     rhs_t[blk:blk + 32, :],
                                     start=(wi == 0), stop=(wi == 2),
                                     tile_position=(blk, 0))
            u = 0
            for t in range(NPAIR):
                if t in p0_pairs:
                    continue
                base = 512 * (u // 4) + NQ * (u % 4)
                blk = 32 * (t // 4)
                nc.tensor.matmul(pt1[:, base:base + NQ],
                                 pv[t % 4][blk:blk + 32, :],
                                 q[blk:blk + 32, :],
                                 start=True, stop=True,
                                 tile_position=(blk, 0))
                u += 1

            # ---- ln(1 + psum), accumulated per partition; asymmetric
            # passes so pass 0 starts after only N0 pairs of matmuls.
            # Throwaway ln outputs land in spare PSUM (access init 172 cyc
            # < SBUF 222). ----
            # Ln pass 0's throwaway output goes to SBUF; pass 1's overwrites
            # the then-dead pt0 banks (PSUM budget: N0+NB1 banks + warm = 8)
            partials = io.tile([128, NPASS], F32, tag="part")
            sout0 = io.tile([128, N0 * NQ], BF16, tag="sout0")
            nc.scalar.activation(
                sout0.rearrange("p (b x) -> p b x", x=NQ),
                pt0.rearrange("p (b x) -> p b x", x=512)[:, :, 0:NQ],
                LN, bias=1.0, accum_out=partials[:, 0:1])
            nc.scalar.activation(
                pt0[:, 0:(NPAIR - N0) * NQ].rearrange(
                    "p (b x) -> p b x", x=4 * NQ),
                pt1.rearrange("p (b x) -> p b x", x=512)[:, :, 0:4 * NQ],
                LN, bias=1.0, accum_out=partials[:, 1:2])

            nc.sync.dma_start(out=y[rep], in_=partials)

    nc.finalize()
    _PROG_CACHE[nreps] = (nc, ())
    return nc, ()


def _consts():
    cs = np.zeros((128, 16), dtype=np.float32)
    p = np.arange(128)
    k = p % 4
    cs[:, 0] = -(1.0 + k)
    for i in range(4):
        cs[:, 8 + i] = (k == i).astype(np.float32)
    return cs


def make_in_maps(prediction, target, consts):
    # fold validity into the prediction: invalid positives -> +50
    # (F^k = e^{-50k} -> 0), invalid negatives -> -50 (E = e^-50 -> 0)
    fill = np.empty((1, N), np.float32)
    fill[:, 0:K] = 50.0
    fill[:, K:N] = -50.0
    pred_m = np.where(target == -1, fill, prediction).astype(np.float32)
    csv = _consts()
    in_maps = []
    for c in range(NC):
        blk = pred_m[c * RPC:(c + 1) * RPC]
        negs = np.ascontiguousarray(blk[:, K:N]).astype(ml_dtypes.bfloat16)
        # doubled positives with the pair-interleave AND pair-of-block
        # masks folded in: posd4[r, v, 64h:64h+64] = pos row r if
        # (h == r%2 and (r%8)//2 == v) else +50
        posd4 = np.full((RPC, 4, 2, K), 50.0, np.float32)
        rr = np.arange(RPC)
        posd4[rr, (rr % 8) // 2, rr % 2, :] = blk[:, 0:K]
        in_maps.append({
            "negs": negs,
            "posd": posd4.reshape(RPC, 4 * 2 * K).astype(ml_dtypes.bfloat16),
            "consts": csv,
        })
    return in_maps


def kernel(prediction, target):
    nc, consts = build_program(1)
    in_maps = make_in_maps(prediction, target, consts)
    res = run_bass_kernel_spmd(nc, in_maps, core_ids=list(range(NC)))
    total = sum(float(res.results[c]["y"][0].sum(dtype=np.float64))
                for c in range(NC))
    return np.float32(total / B)


# revision 25
# speedup vs baseline: 1.2590x; 1.0155x over previous
"""BPR pairwise softplus loss on 8 Trainium2 NeuronCores.

loss = mean_b sum_{i<K, j>=K, both valid} softplus(pred[b,j] - pred[b,i])

Strategy (data parallel over batch, 32 rows/core), folding FOUR negatives
per ln via elementary symmetric polynomials:

  prod_{m=1..4} (1 + F*E_m) = 1 + F*c1 + F^2*c2 + F^3*c3 + F^4*c4
  =>  sum_m softplus(n_m - p) = ln(1 + sum_k F^k c_k),  F = exp(-p), E = exp(n)

Pack-free layout: partition = 4*r + k (row-major, power k innermost), so the
8 contraction partitions of row pair t = (2t, 2t+1) are the contiguous range
8t..8t+8 and the matmul reads the power tile P and coefficient tile q
DIRECTLY - no SBUF->SBUF pack DMAs (each DMA hop costs ~2.7us of fixed
latency: 565 SEQ + 625 HWDGE + 650 engine delay + transfer + 900 sem).

  - P_v[4r+k, 64h+p] = F^{k+1}[r, p] masked to half h == r%2 AND to pairs
    with (r//2)%4 == v: each P_v is ONE ScalarE exp with per-partition
    scale -(k+1) / bias 0 on live partitions and scale 0 / bias -100
    (exp -> 0) elsewhere; the column-interleave mask is folded into the
    host data (+50 fill -> exp(-(k+1)*50) = 0).
  - q[4r+k, j] = c_{k+1}[r, j]: pair folds a/m then masked placement with
    per-partition 0/1 scalar masks via scalar_tensor_tensor:
      q = M1*(a1+a2) + M2*(m1+m2+a1*a2) + M3*(a1*m2+a2*m1) + M4*(m1*m2)
  - 16 matmuls (contraction 32 = one aligned block of 4 pairs, lhsT P_v
    zeroing the other 3 pairs, free 112) straight into PSUM; asymmetric Ln
    passes (N0 pairs then 16-N0) with accum_out row sums; one output DMA.
    (PE tiling: operands must sit at partition base 0/32/64/96 with an
    explicit tile_position, hence the 32-block contraction.)
  - Dummy warm matmuls from ~1us keep the PE p-state ramping so the real
    matmuls run at full clock (a PE idle gap resets the 3us ramp).

Invalid slots (target == -1) fold into the prediction on the host: invalid
positives -> +50 (F^k -> 0), invalid negatives -> -50 (E -> 0).
"""
import sys

sys.path.insert(0, "/opt/trn_rl_repo")

import numpy as np
import ml_dtypes

import concourse.bass as bass
import concourse.mybir as mybir
from concourse import bacc
import concourse.hw_specs as hw_specs
from concourse.tile import TileContext
from concourse.bass_utils import run_bass_kernel_spmd

B, N, K = 256, 512, 64
NC = 8
RPC = B // NC            # 32 batch rows per core
NPAIR = RPC // 2         # 16 row pairs (2t, 2t+1)
NEG = N - K              # 448 negatives per row
G = 4                    # negatives folded per ln
NQ = NEG // G            # 112 quad groups per row
NPASS = 2                # Ln passes
N0 = 4                   # row pairs in Ln pass 0 (rest in pass 1); multiple
                         # of 4 so each pass covers whole PSUM banks
N_WARM = 440             # PE clock warm-up dummy matmuls
WARM_COLS = 16           # free size of each warm matmul

_PROG_CACHE = {}

EXP = mybir.ActivationFunctionType.Exp
LN = mybir.ActivationFunctionType.Ln
F32 = mybir.dt.float32
BF16 = mybir.dt.bfloat16


def _patch_act_tables():
    """Make natural_log_exp_and_others the only table set advertising exp/ln
    so Bacc's table-load pass emits a single ACT_TABLE_LOAD."""
    if getattr(hw_specs.get_activation_tables, "_bpr_patched", False):
        return
    orig_fn = hw_specs.get_activation_tables

    def patched(arch):
        d = orig_fn(arch)
        out = {}
        for name, funcs in d.items():
            if name != "natural_log_exp_and_others" and (EXP in funcs
                                                         or LN in funcs):
                funcs = funcs - {EXP, LN}
            out[name] = funcs
        return out

    patched._bpr_patched = True
    hw_specs.get_activation_tables = patched
    bacc.get_activation_tables = patched


def build_program(nreps: int = 1):
    if nreps in _PROG_CACHE:
        return _PROG_CACHE[nreps]
    _patch_act_tables()
    nc = bacc.Bacc("TRN2", target_bir_lowering=False, debug=False,
                   num_devices=NC)
    negs_d = nc.dram_tensor("negs", [RPC, NEG], BF16, kind="ExternalInput")
    # pre-masked doubled positives, replicated 4x over the pair-of-block
    # index v with the v-mask baked in on the host:
    # posd4[r, v, 64h+p] = pred_pos[r,p] if (h==r%2 and (r%8)//2==v) else +50
    posd_d = nc.dram_tensor("posd", [RPC, 4 * 2 * K], BF16,
                            kind="ExternalInput")
    # col 0: P exp scale -(1+p%4); cols 8-11: one-hot coefficient masks
    # M1..M4 (p%4 == k)
    consts_d = nc.dram_tensor("consts", [128, 16], F32, kind="ExternalInput")
    y = nc.dram_tensor("y", [nreps, 128, NPASS], F32, kind="ExternalOutput")

    mul = mybir.AluOpType.mult
    add = mybir.AluOpType.add

    from contextlib import ExitStack
    with TileContext(nc) as tc, ExitStack() as st:
        io = st.enter_context(tc.tile_pool(name="io", bufs=1))
        ps = st.enter_context(tc.tile_pool(name="ps", bufs=1, space="PSUM"))

        # Trigger the exp/ln activation-table load ASAP (~1.3us on ScalarE,
        # overlapping the input DMA).
        d0 = io.tile([128, 1], F32, tag="d0")
        nc.vector.memset(d0, 0.0)
        d1 = io.tile([128, 1], BF16, tag="d1")
        nc.scalar.activation(d1, d0, EXP)

        # dummy operands for the PE warm-up chain
        dwr = io.tile([8, WARM_COLS], BF16, tag="dwr")
        nc.vector.memset(dwr, 0.0)

        for rep in range(nreps):
            # consts on the Pool SWDGE queue (parallel with SP inputs)
            cs = io.tile([128, 16], F32, tag="cs")
            nc.gpsimd.dma_start(out=cs, in_=consts_d[:])
            m1 = cs[:, 8:9]
            m2 = cs[:, 9:10]
            m3 = cs[:, 10:11]
            m4 = cs[:, 11:12]

            # inputs, replicated 4x across power slots (partition = 4r+k)
            # via stride-0 DRAM reads; negatives first (they gate the chain)
            negs = io.tile([128, NEG], BF16, tag="negs")
            nc.sync.dma_start(
                out=negs,
                in_=negs_d[:].unsqueeze(1).broadcast_to([RPC, 4, NEG]))
            posd = io.tile([128, 4 * 2 * K], BF16, tag="posd")
            nc.sync.dma_start(
                out=posd,
                in_=posd_d[:].unsqueeze(1).broadcast_to([RPC, 4, 4 * 2 * K]))

            # PE warm-up: keep the clock ramping from ~1us until the real
            # matmuls (psum scratch, no consumers)
            pw = ps.tile([128, WARM_COLS], F32, tag="pw")
            for w in range(N_WARM):
                nc.tensor.matmul(pw[0:WARM_COLS], dwr, dwr,
                                 start=True, stop=True)

            # ---- DVE early window (negs land long before exp finishes):
            # log-space sums so the product folds become ACT exps:
            #   m1 = E1*E2 = exp(n1+n2), m2 = E3*E4 = exp(n3+n4),
            #   v = m1*m2 = exp(n1+n2+n3+n4)
            # f32 sums keep exp() accurate (bf16 sums cost ~3% on exp) ----
            nsum = io.tile([128, 2 * NQ], F32, tag="nsum")
            nc.vector.tensor_tensor(nsum[:, 0:NQ], negs[:, 0:NQ],
                                    negs[:, NQ:2 * NQ], add)
            nc.vector.tensor_tensor(nsum[:, NQ:2 * NQ],
                                    negs[:, 2 * NQ:3 * NQ],
                                    negs[:, 3 * NQ:4 * NQ], add)

            # ---- ScalarE stream: exp chunks C1/C2 over negatives, product
            # folds M/V from the log sums, then the four pair-masked power
            # tiles P_v = exp(posd*scale_v + bias_v) ----
            HC = NEG // 2
            e = io.tile([128, NEG], BF16, tag="e")
            nc.scalar.activation(e[:, 0:HC], negs[:, 0:HC], EXP)
            nc.scalar.activation(e[:, HC:NEG], negs[:, HC:NEG], EXP)
            m = io.tile([128, 2 * NQ], BF16, tag="m")
            nc.scalar.activation(m, nsum, EXP)
            # ONE exp for all four pair-masked power tiles (v-mask baked in
            # the host data, power k in the per-partition scale): P_all's
            # column block v holds P_v
            p_all = io.tile([128, 4 * 2 * K], BF16, tag="Pall")
            nc.scalar.activation(p_all, posd, EXP, scale=cs[:, 0:1])
            pv = [p_all[:, 2 * K * vi:2 * K * (vi + 1)] for vi in range(4)]

            # ---- DVE: additive pair folds ----
            a = io.tile([128, 2 * NQ], BF16, tag="a")
            nc.vector.tensor_tensor(a[:, 0:NQ], e[:, 0:NQ], e[:, NQ:2 * NQ],
                                    add)
            nc.vector.tensor_tensor(a[:, NQ:2 * NQ], e[:, 2 * NQ:3 * NQ],
                                    e[:, 3 * NQ:4 * NQ], add)
            a1 = a[:, 0:NQ]
            a2 = a[:, NQ:2 * NQ]
            mm1 = m[:, 0:NQ]
            mm2 = m[:, NQ:2 * NQ]

            # ---- masked coefficient placement:
            # q[4r+k] = c_{k+1}[r]; Mk are per-partition 0/1 scalars ----
            w = io.tile([128, NQ], BF16, tag="w")
            v = io.tile([128, NQ], BF16, tag="v")
            # two products on Pool (off the DVE critical path); Pool only
            # supports plain tensor_tensor, masking happens in the DVE stt
            # chain below
            nc.gpsimd.tensor_tensor(v, mm1, mm2, mul)
            nc.gpsimd.tensor_tensor(w, a2, mm1, mul)

            s_a = io.tile([128, NQ], BF16, tag="sa")
            t_aa2 = io.tile([128, NQ], BF16, tag="taa2")
            s_m = io.tile([128, NQ], BF16, tag="sm")
            t_am3 = io.tile([128, NQ], BF16, tag="tam3")
            t_v4 = io.tile([128, NQ], BF16, tag="tv4")
            y1 = io.tile([128, NQ], BF16, tag="y1")
            y2 = io.tile([128, NQ], BF16, tag="y2")
            y3 = io.tile([128, NQ], BF16, tag="y3")
            z = io.tile([128, NQ], BF16, tag="z")
            q = io.tile([128, NQ], BF16, tag="q")
            nc.vector.tensor_tensor(s_a, a1, a2, add)
            nc.vector.scalar_tensor_tensor(t_aa2, a1, m2, a2, mul, mul)
            nc.vector.tensor_tensor(s_m, mm1, mm2, add)
            nc.vector.scalar_tensor_tensor(t_am3, mm2, m3, a1, mul, mul)
            nc.vector.tensor_scalar(t_v4, v, m4, None, mul)
            nc.vector.scalar_tensor_tensor(y1, s_a, m1, t_aa2, mul, add)
            nc.vector.scalar_tensor_tensor(y2, s_m, m2, t_am3, mul, add)
            # z is independent of y1/y2 so the tail has a single
            # engine-completion wait (q on y3)
            nc.vector.scalar_tensor_tensor(z, w, m3, t_v4, mul, add)
            nc.vector.tensor_tensor(y3, y1, y2, add)
            nc.vector.tensor_tensor(q, y3, z, add)

            # ---- matmuls: psum[64h+p, j] = sum_k F^{k+1}[2t+h, p] *
            # c_{k+1}[2t+h, j]; contraction is the aligned 32-block of 4
            # pairs, P_{t%4} zeroes the other 3.  psum: 4 pairs per 512-col
            # bank (a matmul output cannot cross a bank boundary).
            #
            # Matmul is linear in rhs, so pass-0 pairs {0,4,8,12} (all v=0,
            # needing only P0 which lands first) accumulate THREE early
            # matmul waves rhs = y1, y2, z — their PSUM is complete before
            # the q tile even exists, pulling the first Ln pass forward.
            # Remaining pairs use a single matmul on q = y3 + z. ----
            NB1 = (NPAIR - N0) // 4
            # each pass-0 pair gets its OWN bank (own 2KB zero region), so
            # the three accumulation waves form independent per-bank groups
            pt0 = ps.tile([128, N0 * 512], F32, tag="ps0")
            pt1 = ps.tile([128, NB1 * 512], F32, tag="ps1")
            p0_pairs = [0, 1, 2, 3]
            for wi, rhs_t in enumerate((y1, y2, z)):
                for i, t in enumerate(p0_pairs):
                    blk = 32 * (t // 4)
                    nc.tensor.matmul(pt0[:, 512 * i:512 * i + NQ],
                                     pv[t % 4][blk:blk + 32, :],
                                     rhs_t[blk:blk + 32, :],
                                     start=(wi == 0), stop=(wi == 2),
                                     tile_position=(blk, 0))
            u = 0
            for t in range(NPAIR):
                if t in p0_pairs:
                    continue
                base = 512 * (u // 4) + NQ * (u % 4)
                blk = 32 * (t // 4)
                nc.tensor.matmul(pt1[:, base:base + NQ],
                                 pv[t % 4][blk:blk + 32, :],
                                 q[blk:blk + 32, :],
                                 start=True, stop=True,
                                 tile_position=(blk, 0))
                u += 1

            # ---- ln(1 + psum), accumulated per partition; asymmetric
            # passes so pass 0 starts after only N0 pairs of matmuls.
            # Throwaway ln outputs land in spare PSUM (access init 172 cyc
            # < SBUF 222). ----
            # Ln pass 0's throwaway output goes to SBUF; pass 1's overwrites
            # the then-dead pt0 banks (PSUM budget: N0+NB1 banks + warm = 8)
            partials = io.tile([128, NPASS], F32, tag="part")
            sout0 = io.tile([128, N0 * NQ], BF16, tag="sout0")
            nc.scalar.activation(
                sout0.rearrange("p (b x) -> p b x", x=NQ),
                pt0.rearrange("p (b x) -> p b x", x=512)[:, :, 0:NQ],
                LN, bias=1.0, accum_out=partials[:, 0:1])
            nc.scalar.activation(
                pt0[:, 0:(NPAIR - N0) * NQ].rearrange(
                    "p (b x) -> p b x", x=4 * NQ),
                pt1.rearrange("p (b x) -> p b x", x=512)[:, :, 0:4 * NQ],
                LN, bias=1.0, accum_out=partials[:, 1:2])

            nc.sync.dma_start(out=y[rep], in_=partials)

    nc.finalize()
    _PROG_CACHE[nreps] = (nc, ())
    return nc, ()


def _consts():
    cs = np.zeros((128, 16), dtype=np.float32)
    p = np.arange(128)
    k = p % 4
    cs[:, 0] = -(1.0 + k)
    for i in range(4):
        cs[:, 8 + i] = (k == i).astype(np.float32)
    return cs


def make_in_maps(prediction, target, consts):
    # fold validity into the prediction: invalid positives -> +50
    # (F^k = e^{-50k} -> 0), invalid negatives -> -50 (E = e^-50 -> 0)
    fill = np.empty((1, N), np.float32)
    fill[:, 0:K] = 50.0
    fill[:, K:N] = -50.0
    pred_m = np.where(target == -1, fill, prediction).astype(np.float32)
    csv = _consts()
    in_maps = []
    for c in range(NC):
        blk = pred_m[c * RPC:(c + 1) * RPC]
        negs = np.ascontiguousarray(blk[:, K:N]).astype(ml_dtypes.bfloat16)
        # doubled positives with the pair-interleave AND pair-of-block
        # masks folded in: posd4[r, v, 64h:64h+64] = pos row r if
        # (h == r%2 and (r%8)//2 == v) else +50
        posd4 = np.full((RPC, 4, 2, K), 50.0, np.float32)
        rr = np.arange(RPC)
        posd4[rr, (rr % 8) // 2, rr % 2, :] = blk[:, 0:K]
        in_maps.append({
            "negs": negs,
            "posd": posd4.reshape(RPC, 4 * 2 * K).astype(ml_dtypes.bfloat16),
            "consts": csv,
        })
    return in_maps


def kernel(prediction, target):
    nc, consts = build_program(1)
    in_maps = make_in_maps(prediction, target, consts)
    res = run_bass_kernel_spmd(nc, in_maps, core_ids=list(range(NC)))
    total = sum(float(res.results[c]["y"][0].sum(dtype=np.float64))
                for c in range(NC))
    return np.float32(total / B)


# revision 26
# speedup vs baseline: 1.2666x; 1.0061x over previous
"""BPR pairwise softplus loss on 8 Trainium2 NeuronCores.

loss = mean_b sum_{i<K, j>=K, both valid} softplus(pred[b,j] - pred[b,i])

Strategy (data parallel over batch, 32 rows/core), folding FOUR negatives
per ln via elementary symmetric polynomials:

  prod_{m=1..4} (1 + F*E_m) = 1 + F*c1 + F^2*c2 + F^3*c3 + F^4*c4
  =>  sum_m softplus(n_m - p) = ln(1 + sum_k F^k c_k),  F = exp(-p), E = exp(n)

Pack-free layout: partition = 4*r + k (row-major, power k innermost), so the
8 contraction partitions of row pair t = (2t, 2t+1) are the contiguous range
8t..8t+8 and the matmul reads the power tile P and coefficient tile q
DIRECTLY - no SBUF->SBUF pack DMAs (each DMA hop costs ~2.7us of fixed
latency: 565 SEQ + 625 HWDGE + 650 engine delay + transfer + 900 sem).

  - P_v[4r+k, 64h+p] = F^{k+1}[r, p] masked to half h == r%2 AND to pairs
    with (r//2)%4 == v: each P_v is ONE ScalarE exp with per-partition
    scale -(k+1) / bias 0 on live partitions and scale 0 / bias -100
    (exp -> 0) elsewhere; the column-interleave mask is folded into the
    host data (+50 fill -> exp(-(k+1)*50) = 0).
  - q[4r+k, j] = c_{k+1}[r, j]: pair folds a/m then masked placement with
    per-partition 0/1 scalar masks via scalar_tensor_tensor:
      q = M1*(a1+a2) + M2*(m1+m2+a1*a2) + M3*(a1*m2+a2*m1) + M4*(m1*m2)
  - 16 matmuls (contraction 32 = one aligned block of 4 pairs, lhsT P_v
    zeroing the other 3 pairs, free 112) straight into PSUM; asymmetric Ln
    passes (N0 pairs then 16-N0) with accum_out row sums; one output DMA.
    (PE tiling: operands must sit at partition base 0/32/64/96 with an
    explicit tile_position, hence the 32-block contraction.)
  - Dummy warm matmuls from ~1us keep the PE p-state ramping so the real
    matmuls run at full clock (a PE idle gap resets the 3us ramp).

Invalid slots (target == -1) fold into the prediction on the host: invalid
positives -> +50 (F^k -> 0), invalid negatives -> -50 (E -> 0).
"""
import sys

sys.path.insert(0, "/opt/trn_rl_repo")

import numpy as np
import ml_dtypes

import concourse.bass as bass
import concourse.mybir as mybir
from concourse import bacc
import concourse.hw_specs as hw_specs
from concourse.tile import TileContext
from concourse.bass_utils import run_bass_kernel_spmd

B, N, K = 256, 512, 64
NC = 8
RPC = B // NC            # 32 batch rows per core
NPAIR = RPC // 2         # 16 row pairs (2t, 2t+1)
NEG = N - K              # 448 negatives per row
G = 4                    # negatives folded per ln
NQ = NEG // G            # 112 quad groups per row
NPASS = 2                # Ln passes
N0 = 4                   # row pairs in Ln pass 0 (rest in pass 1); multiple
                         # of 4 so each pass covers whole PSUM banks
N_WARM = 440             # PE clock warm-up dummy matmuls
WARM_COLS = 16           # free size of each warm matmul

_PROG_CACHE = {}

EXP = mybir.ActivationFunctionType.Exp
LN = mybir.ActivationFunctionType.Ln
F32 = mybir.dt.float32
BF16 = mybir.dt.bfloat16


def _patch_act_tables():
    """Make natural_log_exp_and_others the only table set advertising exp/ln
    so Bacc's table-load pass emits a single ACT_TABLE_LOAD."""
    if getattr(hw_specs.get_activation_tables, "_bpr_patched", False):
        return
    orig_fn = hw_specs.get_activation_tables

    def patched(arch):
        d = orig_fn(arch)
        out = {}
        for name, funcs in d.items():
            if name != "natural_log_exp_and_others" and (EXP in funcs
                                                         or LN in funcs):
                funcs = funcs - {EXP, LN}
            out[name] = funcs
        return out

    patched._bpr_patched = True
    hw_specs.get_activation_tables = patched
    bacc.get_activation_tables = patched


def build_program(nreps: int = 1):
    if nreps in _PROG_CACHE:
        return _PROG_CACHE[nreps]
    _patch_act_tables()
    nc = bacc.Bacc("TRN2", target_bir_lowering=False, debug=False,
                   num_devices=NC)
    negs_d = nc.dram_tensor("negs", [RPC, NEG], BF16, kind="ExternalInput")
    # pre-masked doubled positives, replicated 4x over the pair-of-block
    # index v with the v-mask baked in on the host:
    # posd4[r, v, 64h+p] = pred_pos[r,p] if (h==r%2 and (r%8)//2==v) else +50
    posd_d = nc.dram_tensor("posd", [RPC, 4 * 2 * K], BF16,
                            kind="ExternalInput")
    # col 0: P exp scale -(1+p%4); cols 8-11: one-hot coefficient masks
    # M1..M4 (p%4 == k)
    consts_d = nc.dram_tensor("consts", [128, 16], F32, kind="ExternalInput")
    y = nc.dram_tensor("y", [nreps, 128, NPASS], F32, kind="ExternalOutput")

    mul = mybir.AluOpType.mult
    add = mybir.AluOpType.add

    from contextlib import ExitStack
    with TileContext(nc) as tc, ExitStack() as st:
        io = st.enter_context(tc.tile_pool(name="io", bufs=1))
        ps = st.enter_context(tc.tile_pool(name="ps", bufs=1, space="PSUM"))

        # Trigger the exp/ln activation-table load ASAP (~1.3us on ScalarE,
        # overlapping the input DMA).
        d0 = io.tile([128, 1], F32, tag="d0")
        nc.vector.memset(d0, 0.0)
        d1 = io.tile([128, 1], BF16, tag="d1")
        nc.scalar.activation(d1, d0, EXP)

        # dummy operands for the PE warm-up chain
        dwr = io.tile([8, WARM_COLS], BF16, tag="dwr")
        nc.vector.memset(dwr, 0.0)

        for rep in range(nreps):
            # consts on the Pool SWDGE queue (parallel with SP inputs)
            cs = io.tile([128, 16], F32, tag="cs")
            nc.gpsimd.dma_start(out=cs, in_=consts_d[:])
            m1 = cs[:, 8:9]
            m2 = cs[:, 9:10]
            m3 = cs[:, 10:11]
            m4 = cs[:, 11:12]

            # inputs, replicated 4x across power slots (partition = 4r+k)
            # via stride-0 DRAM reads; negatives first (they gate the chain)
            negs = io.tile([128, NEG], BF16, tag="negs")
            nc.sync.dma_start(
                out=negs,
                in_=negs_d[:].unsqueeze(1).broadcast_to([RPC, 4, NEG]))
            posd = io.tile([128, 4 * 2 * K], BF16, tag="posd")
            nc.sync.dma_start(
                out=posd,
                in_=posd_d[:].unsqueeze(1).broadcast_to([RPC, 4, 4 * 2 * K]))

            # PE warm-up: keep the clock ramping from ~1us until the real
            # matmuls (psum scratch, no consumers)
            pw = ps.tile([128, WARM_COLS], F32, tag="pw")
            for w in range(N_WARM):
                nc.tensor.matmul(pw[0:WARM_COLS], dwr, dwr,
                                 start=True, stop=True)

            # ---- DVE early window (negs land long before exp finishes):
            # log-space sums so the product folds become ACT exps:
            #   m1 = E1*E2 = exp(n1+n2), m2 = E3*E4 = exp(n3+n4),
            #   v = m1*m2 = exp(n1+n2+n3+n4)
            # f32 sums keep exp() accurate (bf16 sums cost ~3% on exp) ----
            nsum = io.tile([128, 2 * NQ], F32, tag="nsum")
            nc.vector.tensor_tensor(nsum[:, 0:NQ], negs[:, 0:NQ],
                                    negs[:, NQ:2 * NQ], add)
            nc.vector.tensor_tensor(nsum[:, NQ:2 * NQ],
                                    negs[:, 2 * NQ:3 * NQ],
                                    negs[:, 3 * NQ:4 * NQ], add)

            # ---- ScalarE stream: exp chunks C1/C2 over negatives, product
            # folds M/V from the log sums, then the four pair-masked power
            # tiles P_v = exp(posd*scale_v + bias_v) ----
            e = io.tile([128, NEG], BF16, tag="e")
            nc.scalar.activation(e, negs, EXP)
            m = io.tile([128, 2 * NQ], BF16, tag="m")
            nc.scalar.activation(m, nsum, EXP)
            # ONE exp for all four pair-masked power tiles (v-mask baked in
            # the host data, power k in the per-partition scale): P_all's
            # column block v holds P_v
            p_all = io.tile([128, 4 * 2 * K], BF16, tag="Pall")
            nc.scalar.activation(p_all, posd, EXP, scale=cs[:, 0:1])
            pv = [p_all[:, 2 * K * vi:2 * K * (vi + 1)] for vi in range(4)]

            # ---- DVE: additive pair folds ----
            a = io.tile([128, 2 * NQ], BF16, tag="a")
            nc.vector.tensor_tensor(a[:, 0:NQ], e[:, 0:NQ], e[:, NQ:2 * NQ],
                                    add)
            nc.vector.tensor_tensor(a[:, NQ:2 * NQ], e[:, 2 * NQ:3 * NQ],
                                    e[:, 3 * NQ:4 * NQ], add)
            a1 = a[:, 0:NQ]
            a2 = a[:, NQ:2 * NQ]
            mm1 = m[:, 0:NQ]
            mm2 = m[:, NQ:2 * NQ]

            # ---- masked coefficient placement:
            # q[4r+k] = c_{k+1}[r]; Mk are per-partition 0/1 scalars ----
            w = io.tile([128, NQ], BF16, tag="w")
            v = io.tile([128, NQ], BF16, tag="v")
            # two products on Pool (off the DVE critical path); Pool only
            # supports plain tensor_tensor, masking happens in the DVE stt
            # chain below
            nc.gpsimd.tensor_tensor(v, mm1, mm2, mul)
            nc.gpsimd.tensor_tensor(w, a2, mm1, mul)

            s_a = io.tile([128, NQ], BF16, tag="sa")
            t_aa2 = io.tile([128, NQ], BF16, tag="taa2")
            s_m = io.tile([128, NQ], BF16, tag="sm")
            t_am3 = io.tile([128, NQ], BF16, tag="tam3")
            t_v4 = io.tile([128, NQ], BF16, tag="tv4")
            y1 = io.tile([128, NQ], BF16, tag="y1")
            y2 = io.tile([128, NQ], BF16, tag="y2")
            y3 = io.tile([128, NQ], BF16, tag="y3")
            z = io.tile([128, NQ], BF16, tag="z")
            q = io.tile([128, NQ], BF16, tag="q")
            nc.vector.tensor_tensor(s_a, a1, a2, add)
            nc.vector.scalar_tensor_tensor(t_aa2, a1, m2, a2, mul, mul)
            nc.vector.tensor_tensor(s_m, mm1, mm2, add)
            nc.vector.scalar_tensor_tensor(t_am3, mm2, m3, a1, mul, mul)
            nc.vector.tensor_scalar(t_v4, v, m4, None, mul)
            nc.vector.scalar_tensor_tensor(y1, s_a, m1, t_aa2, mul, add)
            nc.vector.scalar_tensor_tensor(y2, s_m, m2, t_am3, mul, add)
            # z is independent of y1/y2 so the tail has a single
            # engine-completion wait (q on y3)
            nc.vector.scalar_tensor_tensor(z, w, m3, t_v4, mul, add)
            nc.vector.tensor_tensor(y3, y1, y2, add)
            nc.vector.tensor_tensor(q, y3, z, add)

            # ---- matmuls: psum[64h+p, j] = sum_k F^{k+1}[2t+h, p] *
            # c_{k+1}[2t+h, j]; contraction is the aligned 32-block of 4
            # pairs, P_{t%4} zeroes the other 3.  psum: 4 pairs per 512-col
            # bank (a matmul output cannot cross a bank boundary).
            #
            # Matmul is linear in rhs, so pass-0 pairs {0,4,8,12} (all v=0,
            # needing only P0 which lands first) accumulate THREE early
            # matmul waves rhs = y1, y2, z — their PSUM is complete before
            # the q tile even exists, pulling the first Ln pass forward.
            # Remaining pairs use a single matmul on q = y3 + z. ----
            NB1 = (NPAIR - N0) // 4
            # each pass-0 pair gets its OWN bank (own 2KB zero region), so
            # the three accumulation waves form independent per-bank groups
            pt0 = ps.tile([128, N0 * 512], F32, tag="ps0")
            pt1 = ps.tile([128, NB1 * 512], F32, tag="ps1")
            p0_pairs = [0, 1, 2, 3]
            for wi, rhs_t in enumerate((y1, y2, z)):
                for i, t in enumerate(p0_pairs):
                    blk = 32 * (t // 4)
                    nc.tensor.matmul(pt0[:, 512 * i:512 * i + NQ],
                                     pv[t % 4][blk:blk + 32, :],
                                     rhs_t[blk:blk + 32, :],
                                     start=(wi == 0), stop=(wi == 2),
                                     tile_position=(blk, 0))
            u = 0
            for t in range(NPAIR):
                if t in p0_pairs:
                    continue
                base = 512 * (u // 4) + NQ * (u % 4)
                blk = 32 * (t // 4)
                nc.tensor.matmul(pt1[:, base:base + NQ],
                                 pv[t % 4][blk:blk + 32, :],
                                 q[blk:blk + 32, :],
                                 start=True, stop=True,
                                 tile_position=(blk, 0))
                u += 1

            # ---- ln(1 + psum), accumulated per partition; asymmetric
            # passes so pass 0 starts after only N0 pairs of matmuls.
            # Throwaway ln outputs land in spare PSUM (access init 172 cyc
            # < SBUF 222). ----
            # Ln pass 0's throwaway output goes to SBUF; pass 1's overwrites
            # the then-dead pt0 banks (PSUM budget: N0+NB1 banks + warm = 8)
            partials = io.tile([128, NPASS], F32, tag="part")
            sout0 = io.tile([128, N0 * NQ], BF16, tag="sout0")
            nc.scalar.activation(
                sout0.rearrange("p (b x) -> p b x", x=NQ),
                pt0.rearrange("p (b x) -> p b x", x=512)[:, :, 0:NQ],
                LN, bias=1.0, accum_out=partials[:, 0:1])
            nc.scalar.activation(
                pt0[:, 0:(NPAIR - N0) * NQ].rearrange(
                    "p (b x) -> p b x", x=4 * NQ),
                pt1.rearrange("p (b x) -> p b x", x=512)[:, :, 0:4 * NQ],
                LN, bias=1.0, accum_out=partials[:, 1:2])

            nc.sync.dma_start(out=y[rep], in_=partials)

    nc.finalize()
    _PROG_CACHE[nreps] = (nc, ())
    return nc, ()


def _consts():
    cs = np.zeros((128, 16), dtype=np.float32)
    p = np.arange(128)
    k = p % 4
    cs[:, 0] = -(1.0 + k)
    for i in range(4):
        cs[:, 8 + i] = (k == i).astype(np.float32)
    return cs


def make_in_maps(prediction, target, consts):
    # fold validity into the prediction: invalid positives -> +50
    # (F^k = e^{-50k} -> 0), invalid negatives -> -50 (E = e^-50 -> 0)
    fill = np.empty((1, N), np.float32)
    fill[:, 0:K] = 50.0
    fill[:, K:N] = -50.0
    pred_m = np.where(target == -1, fill, prediction).astype(np.float32)
    csv = _consts()
    in_maps = []
    for c in range(NC):
        blk = pred_m[c * RPC:(c + 1) * RPC]
        negs = np.ascontiguousarray(blk[:, K:N]).astype(ml_dtypes.bfloat16)
        # doubled positives with the pair-interleave AND pair-of-block
        # masks folded in: posd4[r, v, 64h:64h+64] = pos row r if
        # (h == r%2 and (r%8)//2 == v) else +50
        posd4 = np.full((RPC, 4, 2, K), 50.0, np.float32)
        rr = np.arange(RPC)
        posd4[rr, (rr % 8) // 2, rr % 2, :] = blk[:, 0:K]
        in_maps.append({
            "negs": negs,
            "posd": posd4.reshape(RPC, 4 * 2 * K).astype(ml_dtypes.bfloat16),
            "consts": csv,
        })
    return in_maps


def kernel(prediction, target):
    nc, consts = build_program(1)
    in_maps = make_in_maps(prediction, target, consts)
    res = run_bass_kernel_spmd(nc, in_maps, core_ids=list(range(NC)))
    total = sum(float(res.results[c]["y"][0].sum(dtype=np.float64))
                for c in range(NC))
    return np.float32(total / B)
